# revision 11
# baseline (speedup 1.0000x reference)
"""CRNN Trainium2 kernel: patchify-conv -> 3x3 conv -> pool -> GRU encoder ->
autoregressive GRU decoder. Pure data-parallel over batch (32 -> 8 cores x 4).

Host side: numpy layout transforms (patch-transpose of frames, weight
re-layouts, BN/bias folding). Device side: one Bass/Tile SPMD program run on 8
NeuronCores via run_bass_kernel_spmd.
"""

import os
import sys

for _p in ("/opt/trn_rl_repo", "/root/.axon_site/_ro/trn_rl_repo"):
    if os.path.isdir(_p) and _p not in sys.path:
        sys.path.insert(0, _p)

import numpy as np

import concourse.bass as bass  # noqa: E402
import concourse.mybir as mybir  # noqa: E402
import concourse.tile as tile  # noqa: E402
from concourse import bacc  # noqa: E402
from concourse.bass_utils import run_bass_kernel_spmd  # noqa: E402

F32 = mybir.dt.float32
AF = mybir.ActivationFunctionType

# Model dims (hardcoded from the problem spec)
B, L, DS, DA, DC, DR, DO, HOR = 32, 16, 12, 16, 64, 256, 2, 10
NCORES, BPC = 8, 4          # batch per core
NG, FPG = 8, 8              # 8 groups of 8 frames per core (frame idx = l*4+b)
NPOS = 49                   # 7x7 patch grid
BN_EPS = 1e-5

# 'f32r' = single-pass full-rate fp32 matmul (reduced internal precision),
# 'f32'  = exact two-pass fp32. Applied via AP bitcast at matmul sites only.
MM_DT_CONV = os.environ.get("BASS_MM_DT_CONV", "f32r")
MM_DT_RNN = os.environ.get("BASS_MM_DT_RNN", "f32r")

LAST_EXEC_NS = None
LAST_RESULTS = None


def _smalls_layout():
    """Column layout of the packed per-core 'smalls' tensor [128, ncols]."""
    out = {}
    cols = 0

    def add(name, rows, width):
        nonlocal cols
        out[name] = (rows, cols, width)
        cols += width

    add("b2t", 64, 392)      # conv2 bias table (64, 49) tiled x8 frames
    add("pscale", 64, 1)     # pool+BN scale
    add("pshift", 64, 1)     # pool+BN shift
    add("xt", 12, 64)        # per-core x transposed, col = l*4+b
    add("a0t", 12, 16)
    add("a0b", 16, 1)
    add("ait", 16, 16)
    add("aib", 16, 1)
    add("ants2", 16, 256)    # an_w[:, :16].T
    add("antf", 64, 256)     # an_w[:, 16:].T
    add("anb", 128, 2)       # an_b chunks as cols
    add("bsgi", 128, 6)      # b_ih + b_hh (rz) / b_ih (n), chunk cols
    add("bsdec", 128, 24)    # same, tiled x4 batch cols
    add("bhhn", 128, 8)      # b_hh n-part, tiled x4
    add("fib", 128, 8)       # fi_b chunk cols tiled x4
    add("fnb", 2, 1)
    return out, cols


SM_LAYOUT, SM_COLS = _smalls_layout()

# conv2 shifts: center first so its start=True initializes the full PSUM rect.
SHIFTS = [(1, 1)] + [(dh, dw) for dh in range(3) for dw in range(3)
                     if (dh, dw) != (1, 1)]


def build_nc():
    nc = bacc.Bacc("TRN2", target_bir_lowering=False, debug=False,
                   num_devices=NCORES)

    h_fr = nc.dram_tensor("fr", [NG, 128, 6 * 392], F32, kind="ExternalInput")
    h_sm = nc.dram_tensor("smalls", [128, SM_COLS], F32, kind="ExternalInput")
    h_w1 = nc.dram_tensor("w1", [128, 6 * 576], F32, kind="ExternalInput")
    h_w2 = nc.dram_tensor("w2", [128, 45 * 64], F32, kind="ExternalInput")
    h_wih = nc.dram_tensor("wih", [128, 2 * 768], F32, kind="ExternalInput")
    h_whh = nc.dram_tensor("whh", [128, 2 * 768], F32, kind="ExternalInput")
    h_fi = nc.dram_tensor("fiw", [128, 2 * 256], F32, kind="ExternalInput")
    h_fn = nc.dram_tensor("fnw", [128, 4], F32, kind="ExternalInput")
    h_out = nc.dram_tensor("out", [2, 4 * HOR], F32, kind="ExternalOutput")
    debug = bool(int(os.environ.get("KERNEL_DEBUG", "0")))
    if debug:
        h_dsenc = nc.dram_tensor("d_senc", [128, 2, 64], F32,
                                 kind="ExternalOutput")
        h_dgi = nc.dram_tensor("d_gi", [128, 6, 64], F32,
                               kind="ExternalOutput")
        h_dh = nc.dram_tensor("d_h", [128, 8], F32, kind="ExternalOutput")
        h_df1 = nc.dram_tensor("d_f1", [128, 5, FPG, 9, 9], F32,
                               kind="ExternalOutput")
        h_dpg0 = nc.dram_tensor("d_pg0", [128, 48], F32,
                                kind="ExternalOutput")
        h_dhn0 = nc.dram_tensor("d_hn0", [128, 8], F32,
                                kind="ExternalOutput")
        h_dxr0 = nc.dram_tensor("d_xr0", [128, 8], F32,
                                kind="ExternalOutput")

    mm_conv = mybir.dt.float32r if MM_DT_CONV == "f32r" else F32
    mm_rnn = mybir.dt.float32r if MM_DT_RNN == "f32r" else F32

    def mmc(out, lhsT, rhs, **kw):
        nc.tensor.matmul(out, lhsT.bitcast(mm_conv), rhs.bitcast(mm_conv),
                         skip_group_check=True, **kw)

    def mmr(out, lhsT, rhs, **kw):
        nc.tensor.matmul(out, lhsT.bitcast(mm_rnn), rhs.bitcast(mm_rnn),
                         skip_group_check=True, **kw)

    with tile.TileContext(nc) as tc:
        from contextlib import ExitStack
        with ExitStack() as ctx:
            cpool = ctx.enter_context(tc.tile_pool(name="const", bufs=1))
            xin_pool = ctx.enter_context(tc.tile_pool(name="xin", bufs=3))
            f1_pool = ctx.enter_context(tc.tile_pool(name="f1", bufs=2))
            work = ctx.enter_context(tc.tile_pool(name="work", bufs=4))
            state = ctx.enter_context(tc.tile_pool(name="state", bufs=1))
            hpool = ctx.enter_context(tc.tile_pool(name="h", bufs=3))
            ps1 = ctx.enter_context(
                tc.tile_pool(name="ps1", bufs=4, space="PSUM"))
            ps2 = ctx.enter_context(
                tc.tile_pool(name="ps2", bufs=1, space="PSUM"))
            psr = ctx.enter_context(
                tc.tile_pool(name="psr", bufs=3, space="PSUM"))

            # ---- constants to SBUF ----
            w1 = cpool.tile([128, 6 * 576], F32, tag="w1")
            nc.sync.dma_start(w1[:], h_w1[:])
            w2 = cpool.tile([128, 45 * 64], F32, tag="w2")
            nc.sync.dma_start(w2[:], h_w2[:])
            wih = cpool.tile([128, 2 * 768], F32, tag="wih")
            nc.sync.dma_start(wih[:], h_wih[:])
            whh = cpool.tile([128, 2 * 768], F32, tag="whh")
            nc.sync.dma_start(whh[:], h_whh[:])
            fiw = cpool.tile([128, 2 * 256], F32, tag="fiw")
            nc.sync.dma_start(fiw[:], h_fi[:])
            fnw = cpool.tile([128, 4], F32, tag="fnw")
            nc.sync.dma_start(fnw[:], h_fn[:])
            sm = cpool.tile([128, SM_COLS], F32, tag="sm")
            nc.sync.dma_start(sm[:], h_sm[:])

            def sv(name):
                rows, off, width = SM_LAYOUT[name]
                return sm[0:rows, off:off + width]

            def svc(name, c0, w):  # column sub-slice of a smalls entry
                rows, off, width = SM_LAYOUT[name]
                assert c0 + w <= width
                return sm[0:rows, off + c0:off + c0 + w]

            # ---- persistent state tiles ----
            s2 = state.tile([16, 64], F32, tag="s2")
            s_enc = state.tile([128, 2, 64], F32, tag="senc")
            GI = state.tile([128, 6, 64], F32, tag="gi")
            preds = state.tile([2, 4 * HOR], F32, tag="preds")

            # ---- state adapters: s1 = relu(a0 x); s2 = s1 + relu(ai s1) ----
            pa = psr.tile([16, 64], F32, tag="ps")
            mmr(pa[:], sv("a0t"), sv("xt"), start=True, stop=True)
            s1 = work.tile([16, 64], F32, tag="s1")
            nc.scalar.activation(s1[:], pa[:], AF.Relu, bias=sv("a0b"))
            pb = psr.tile([16, 64], F32, tag="ps")
            mmr(pb[:], sv("ait"), s1[:], start=True, stop=True)
            s1b = work.tile([16, 64], F32, tag="s1")
            nc.scalar.activation(s1b[:], pb[:], AF.Relu, bias=sv("aib"))
            nc.vector.tensor_add(s2[:], s1[:], s1b[:])

            # encoder hidden state
            h_cur = hpool.tile([128, 8], F32, tag="h")
            nc.vector.memset(h_cur[:], 0.0)

            def enc_step(t, h_prev):
                pg = psr.tile([128, 24], F32, tag="ps")
                for mc in range(6):
                    for kc in range(2):
                        mmr(pg[:, mc * 4:(mc + 1) * 4],
                            whh[:, kc * 768 + mc * 128:kc * 768 + (mc + 1) * 128],
                            h_prev[:, kc * 4:(kc + 1) * 4],
                            start=(kc == 0), stop=(kc == 1))
                gi_rz = GI[:, 0:4, t * 4:(t + 1) * 4]
                gi_n = GI[:, 4:6, t * 4:(t + 1) * 4]
                pre = work.tile([128, 16], F32, tag="g16")
                nc.vector.tensor_add(
                    pre[:].rearrange("p (c b) -> p c b", b=4), gi_rz,
                    pg[:, 0:16].rearrange("p (c b) -> p c b", b=4))
                rz = work.tile([128, 16], F32, tag="g16b")
                nc.scalar.activation(rz[:], pre[:], AF.Sigmoid)
                a1 = work.tile([128, 8], F32, tag="g8")
                nc.vector.tensor_add(a1[:], pg[:, 16:24], sv("bhhn"))
                a2 = work.tile([128, 8], F32, tag="g8b")
                nc.vector.tensor_mul(a2[:], rz[:, 0:8], a1[:])
                a3 = work.tile([128, 8], F32, tag="g8c")
                nc.vector.tensor_add(
                    a3[:].rearrange("p (c b) -> p c b", b=4),
                    a2[:].rearrange("p (c b) -> p c b", b=4), gi_n)
                nt = work.tile([128, 8], F32, tag="g8d")
                nc.scalar.activation(nt[:], a3[:], AF.Tanh)
                hmn = work.tile([128, 8], F32, tag="g8e")
                nc.vector.tensor_sub(hmn[:], h_prev[:], nt[:])
                zt = work.tile([128, 8], F32, tag="g8f")
                nc.vector.tensor_mul(zt[:], rz[:, 8:16], hmn[:])
                h_new = hpool.tile([128, 8], F32, tag="h")
                nc.vector.tensor_add(h_new[:], nt[:], zt[:])
                return h_new

            # ---- conv + features + GI, per group of 8 frames ----
            for g in range(NG):
                xin = xin_pool.tile([128, 6 * 392], F32, tag="xin")
                nc.sync.dma_start(xin[:], h_fr[g])

                # f1 is halo-padded to 9x9 per frame; halo + K-pad rows are
                # zeroed so every conv2 shift reads a full valid 7x7 window.
                f1 = f1_pool.tile([128, 5, FPG, 9, 9], F32, tag="f1")
                nc.gpsimd.memset(f1[:], 0.0)

                for m in range(5):
                    msz = 128 if m < 4 else 64
                    p1 = ps1.tile([msz, 392], F32, tag="c1")
                    for k in range(6):
                        mmc(p1[:],
                            w1[:, k * 576 + m * 128:k * 576 + m * 128 + msz],
                            xin[:, k * 392:(k + 1) * 392],
                            start=(k == 0), stop=(k == 5))
                    nc.vector.tensor_copy(
                        f1[0:msz, m, :, 1:8, 1:8],
                        p1[:].rearrange("p (f a b) -> p f a b", a=7, b=7))

                p2 = ps2.tile([64, FPG * 49], F32, tag="c2")
                for si, (dh, dw) in enumerate(SHIFTS):
                    s = dh * 3 + dw
                    for k in range(5):
                        rhs = f1[:, k, :, dh:dh + 7, dw:dw + 7]
                        mmc(p2[:],
                            w2[:, (s * 5 + k) * 64:(s * 5 + k + 1) * 64],
                            rhs, start=(si == 0 and k == 0),
                            stop=(si == len(SHIFTS) - 1 and k == 4))

                # epilogue: relu(conv2 + B2) -> mean -> BN affine
                t0 = work.tile([64, FPG * 49], F32, tag="ep")
                nc.vector.tensor_add(t0[:], p2[:], sv("b2t"))
                t1 = work.tile([64, FPG * 49], F32, tag="ep2")
                nc.vector.tensor_scalar_max(t1[:], t0[:], 0.0)
                red = work.tile([64, FPG], F32, tag="red")
                nc.vector.tensor_reduce(
                    red[:], t1[:].rearrange("p (f s) -> p f s", s=49),
                    axis=mybir.AxisListType.X, op=mybir.AluOpType.add)
                feats = work.tile([64, FPG], F32, tag="feats")
                nc.scalar.activation(feats[:], red[:], AF.Identity,
                                     bias=sv("pshift"), scale=sv("pscale"))

                # an: relu(an_w [s2; feats] + an_b), K split 64(feats)+16(s2)
                gcol = slice(g * FPG, (g + 1) * FPG)
                for mc in range(2):
                    pan = psr.tile([128, FPG], F32, tag="ps")
                    mmr(pan[:], svc("antf", mc * 128, 128), feats[:],
                        start=True, stop=False)
                    mmr(pan[:], svc("ants2", mc * 128, 128), s2[:, gcol],
                        start=False, stop=True)
                    nc.scalar.activation(s_enc[:, mc, gcol], pan[:], AF.Relu,
                                         bias=svc("anb", mc, 1))

                # GI = w_ih @ s_enc + (b_ih + b_hh fold) for these 8 cols
                for mc in range(6):
                    pgi = psr.tile([128, FPG], F32, tag="ps")
                    for kc in range(2):
                        mmr(pgi[:],
                            wih[:, kc * 768 + mc * 128:kc * 768 + (mc + 1) * 128],
                            s_enc[:, kc, gcol],
                            start=(kc == 0), stop=(kc == 1))
                    nc.scalar.activation(GI[:, mc, gcol], pgi[:], AF.Identity,
                                         bias=svc("bsgi", mc, 1))

                if debug and g == 0:
                    nc.sync.dma_start(h_df1[:], f1[:])

                # encoder steps that become ready after this group
                h_cur = enc_step(2 * g, h_cur)
                h_cur = enc_step(2 * g + 1, h_cur)

            if debug:
                nc.sync.dma_start(h_dsenc[:], s_enc[:])
                nc.sync.dma_start(h_dgi[:], GI[:])
                nc.sync.dma_start(h_dh[:], h_cur[:])

            # ---- decoder ----
            xi, hh = h_cur, h_cur
            for t in range(HOR):
                pg = psr.tile([128, 48], F32, tag="ps")
                # each PSUM accumulation group must complete before the next
                # one starts (interleaved groups break accumulation)
                for mc in range(6):
                    for kc in range(2):
                        mmr(pg[:, mc * 4:(mc + 1) * 4],
                            wih[:, kc * 768 + mc * 128:
                                kc * 768 + (mc + 1) * 128],
                            xi[:, kc * 4:(kc + 1) * 4],
                            start=(kc == 0), stop=(kc == 1))
                    for kc in range(2):
                        mmr(pg[:, 24 + mc * 4:24 + (mc + 1) * 4],
                            whh[:, kc * 768 + mc * 128:
                                kc * 768 + (mc + 1) * 128],
                            hh[:, kc * 4:(kc + 1) * 4],
                            start=(kc == 0), stop=(kc == 1))
                if debug and t == 0:
                    pgc = work.tile([128, 48], F32, tag="dbgpg")
                    nc.vector.tensor_copy(pgc[:], pg[:])
                    nc.sync.dma_start(h_dpg0[:], pgc[:])
                gisb = work.tile([128, 24], F32, tag="g24")
                nc.vector.tensor_add(gisb[:], pg[:, 0:24], sv("bsdec"))
                pre = work.tile([128, 16], F32, tag="g16")
                nc.vector.tensor_add(pre[:], gisb[:, 0:16], pg[:, 24:40])
                rz = work.tile([128, 16], F32, tag="g16b")
                nc.scalar.activation(rz[:], pre[:], AF.Sigmoid)
                a1 = work.tile([128, 8], F32, tag="g8")
                nc.vector.tensor_add(a1[:], pg[:, 40:48], sv("bhhn"))
                a2 = work.tile([128, 8], F32, tag="g8b")
                nc.vector.tensor_mul(a2[:], rz[:, 0:8], a1[:])
                a3 = work.tile([128, 8], F32, tag="g8c")
                nc.vector.tensor_add(a3[:], a2[:], gisb[:, 16:24])
                nt = work.tile([128, 8], F32, tag="g8d")
                nc.scalar.activation(nt[:], a3[:], AF.Tanh)
                hmn = work.tile([128, 8], F32, tag="g8e")
                nc.vector.tensor_sub(hmn[:], hh[:], nt[:])
                zt = work.tile([128, 8], F32, tag="g8f")
                nc.vector.tensor_mul(zt[:], rz[:, 8:16], hmn[:])
                hn = hpool.tile([128, 8], F32, tag="h")
                nc.vector.tensor_add(hn[:], nt[:], zt[:])

                pfi = psr.tile([128, 8], F32, tag="ps")
                for mc2 in range(2):
                    for kc2 in range(2):
                        mmr(pfi[:, mc2 * 4:(mc2 + 1) * 4],
                            fiw[:, kc2 * 256 + mc2 * 128:
                                kc2 * 256 + (mc2 + 1) * 128],
                            hn[:, kc2 * 4:(kc2 + 1) * 4],
                            start=(kc2 == 0), stop=(kc2 == 1))
                u1 = work.tile([128, 8], F32, tag="g8")
                nc.vector.tensor_add(u1[:], pfi[:], sv("fib"))
                u2 = work.tile([128, 8], F32, tag="g8b")
                nc.vector.tensor_scalar_max(u2[:], u1[:], 0.0)
                xr = hpool.tile([128, 8], F32, tag="xr")
                nc.vector.tensor_add(xr[:], hn[:], u2[:])

                pfn = psr.tile([2, 4], F32, tag="ps")
                for kc in range(2):
                    mmr(pfn[:], fnw[:, kc * 2:(kc + 1) * 2],
                        xr[:, kc * 4:(kc + 1) * 4],
                        start=(kc == 0), stop=(kc == 1))
                nc.scalar.activation(preds[:, t * 4:(t + 1) * 4], pfn[:],
                                     AF.Tanh, bias=sv("fnb"))
                if debug and t == 0:
                    nc.sync.dma_start(h_dhn0[:], hn[:])
                    nc.sync.dma_start(h_dxr0[:], xr[:])
                xi, hh = xr, hn

            nc.sync.dma_start(h_out[:], preds[:])

    nc.finalize()
    return nc


# ---------------- host-side data prep ----------------

def _prep_frames(frames):
    """frames (32,16,3,112,112) -> per-core [NG, 128, 2352] patch-T layout."""
    out = np.empty((NCORES, NG, 128, 6 * 392), np.float32)
    fr = np.ascontiguousarray(frames, np.float32)
    for c in range(NCORES):
        fb = fr[c * BPC:(c + 1) * BPC]  # (4, 16, 3, 112, 112)
        a = fb.reshape(BPC, L, 3, 7, 16, 7, 16)
        # -> [l, b, ch, kh, kw, ph, pw]
        a = a.transpose(1, 0, 2, 4, 6, 3, 5)
        a = a.reshape(L, BPC, 768, 49)
        a = a.reshape(NG, 2, BPC, 6, 128, 49)
        # -> [g, k, p, li, b, s]
        a = a.transpose(0, 3, 4, 1, 2, 5)
        a = a.reshape(NG, 6, 128, 392)
        a = a.transpose(0, 2, 1, 3)  # [g, p, k, 392]
        out[c] = a.reshape(NG, 128, 6 * 392)
    return out


def _prep_weights(iv):
    w = {}
    W1f = iv["cnn_w"].reshape(576, 768).astype(np.float32)
    w["w1"] = np.ascontiguousarray(
        W1f.T.reshape(6, 128, 576).transpose(1, 0, 2).reshape(128, 6 * 576))

    w2h = np.zeros((9, 5, 128, 64), np.float32)
    for dh in range(3):
        for dw in range(3):
            s = dh * 3 + dw
            T = iv["cnn1_w"][:, :, dh, dw].T.astype(np.float32)  # (576, 64)
            Tp = np.zeros((640, 64), np.float32)
            Tp[:576] = T
            w2h[s] = Tp.reshape(5, 128, 64)
    w["w2"] = np.ascontiguousarray(
        w2h.transpose(2, 0, 1, 3).reshape(128, 45 * 64))

    for name, key in (("wih", "w_ih"), ("whh", "w_hh")):
        T = iv[key].T.astype(np.float32)  # (256, 768)
        w[name] = np.ascontiguousarray(
            T.reshape(2, 128, 768).transpose(1, 0, 2).reshape(128, 1536))
    T = iv["fi_w"].T.astype(np.float32)  # (256, 256)
    w["fiw"] = np.ascontiguousarray(
        T.reshape(2, 128, 256).transpose(1, 0, 2).reshape(128, 512))
    T = iv["fn_w"].T.astype(np.float32)  # (256, 2)
    w["fnw"] = np.ascontiguousarray(
        T.reshape(2, 128, 2).transpose(1, 0, 2).reshape(128, 4))
    return w


def _prep_smalls(iv, x, core):
    sm = np.zeros((128, SM_COLS), np.float32)

    def put(name, arr):
        rows, off, width = SM_LAYOUT[name]
        a = np.asarray(arr, np.float32).reshape(rows, width)
        sm[0:rows, off:off + width] = a

    # conv2 position-dependent bias fold (conv1 bias + cnn1_b)
    M = np.einsum("oiab,i->oab", iv["cnn1_w"], iv["cnn_b"]).astype(np.float32)
    B2 = np.zeros((64, 7, 7), np.float32)
    for ph in range(7):
        for pw in range(7):
            acc = iv["cnn1_b"].astype(np.float32).copy()
            for dh in range(3):
                for dw in range(3):
                    if 0 <= ph + dh - 1 <= 6 and 0 <= pw + dw - 1 <= 6:
                        acc = acc + M[:, dh, dw]
            B2[:, ph, pw] = acc
    put("b2t", np.tile(B2.reshape(64, 49), (1, FPG)))

    inv = iv["bn_g"] / np.sqrt(iv["bn_v"] + BN_EPS)
    put("pscale", (inv / 49.0)[:, None])
    put("pshift", (iv["bn_b"] - iv["bn_m"] * inv)[:, None])

    xb = x[core * BPC:(core + 1) * BPC]  # (4, 16, 12)
    put("xt", xb.transpose(2, 1, 0).reshape(12, 64))

    put("a0t", iv["a0_w"].T)
    put("a0b", iv["a0_b"][:, None])
    put("ait", iv["ai_w"].T)
    put("aib", iv["ai_b"][:, None])
    put("ants2", iv["an_w"][:, 0:16].T)
    put("antf", iv["an_w"][:, 16:80].T)
    put("anb", iv["an_b"].reshape(2, 128).T)

    bs = (iv["b_ih"] + iv["b_hh"]).astype(np.float32)
    bs[512:] = iv["b_ih"][512:]
    put("bsgi", bs.reshape(6, 128).T)
    put("bsdec", np.repeat(bs.reshape(6, 128).T, 4, axis=1))
    put("bhhn", np.repeat(iv["b_hh"][512:].reshape(2, 128).T, 4, axis=1))
    put("fib", np.repeat(iv["fi_b"].reshape(2, 128).T, 4, axis=1))
    put("fnb", iv["fn_b"][:, None])
    return sm


def make_in_maps(inputs):
    iv = {k: np.asarray(v, np.float32) for k, v in inputs.items()}
    frames = iv["frames"]
    x = iv["x"]
    fr_all = _prep_frames(frames)
    w = _prep_weights(iv)
    in_maps = []
    for c in range(NCORES):
        m = {"fr": np.ascontiguousarray(fr_all[c]),
             "smalls": _prep_smalls(iv, x, c)}
        m.update(w)
        in_maps.append(m)
    return in_maps


_NC_CACHE = None


def get_nc():
    global _NC_CACHE
    if _NC_CACHE is None:
        _NC_CACHE = build_nc()
    return _NC_CACHE


def _install_ntff_hook():
    """The agent image's antenv lacks axon_hooks; synthesize it so
    run_bass_kernel_spmd(trace=True) can capture NTFF profiles."""
    try:
        from antenv.axon_hooks import get_axon_ntff_profile_hook  # noqa: F401
        return True
    except ImportError:
        pass
    try:
        import types
        import antenv
        if "/root/.axon_site" not in sys.path:
            sys.path.insert(0, "/root/.axon_site")
        from trn_agent_boot.trn_boot import _ntff_profile_via_ctypes
        hook = _ntff_profile_via_ctypes("/opt/axon/libaxon_pjrt.so")
        mod = types.ModuleType("antenv.axon_hooks")
        mod.get_axon_ntff_profile_hook = lambda: hook
        mod.set_axon_ntff_profile_hook = lambda h: None
        sys.modules["antenv.axon_hooks"] = mod
        antenv.axon_hooks = mod
        return hook is not None
    except Exception as e:  # pragma: no cover - profiling is best-effort
        print(f"ntff hook install failed: {e}")
        return False


def kernel(**inputs):
    global LAST_EXEC_NS, LAST_RESULTS
    nc = get_nc()
    in_maps = make_in_maps(inputs)
    trace = bool(int(os.environ.get("KERNEL_TRACE", "0")))
    if trace:
        trace = _install_ntff_hook()
    res = run_bass_kernel_spmd(nc, in_maps, core_ids=list(range(NCORES)),
                               trace=trace)
    LAST_RESULTS = res
    LAST_EXEC_NS = res.exec_time_ns
    outs = []
    for c in range(NCORES):
        o = res.results[c]["out"]  # (2, 40)
        outs.append(o.reshape(2, HOR, BPC).transpose(1, 2, 0)[:, :, None, :])
    return np.concatenate(outs, axis=1).astype(np.float32)


if __name__ == "__main__":
    nc = get_nc()
    print("built ok; instructions:",
          sum(len(bb.instructions) for bb in nc.main_func.blocks))


# revision 17
# speedup vs baseline: 1.6280x; 1.6280x over previous
"""CRNN Trainium2 kernel: patchify-conv -> 3x3 conv -> pool -> GRU encoder ->
autoregressive GRU decoder. Pure data-parallel over batch (32 -> 8 cores x 4).

Host side: numpy layout transforms (patch-transpose of frames, weight
re-layouts, BN/bias folding). Device side: one Bass/Tile SPMD program run on 8
NeuronCores via run_bass_kernel_spmd.
"""

import os
import sys

for _p in ("/opt/trn_rl_repo", "/root/.axon_site/_ro/trn_rl_repo"):
    if os.path.isdir(_p) and _p not in sys.path:
        sys.path.insert(0, _p)

import numpy as np

import concourse.bass as bass  # noqa: E402
import concourse.mybir as mybir  # noqa: E402
import concourse.tile as tile  # noqa: E402
from concourse import bacc  # noqa: E402
from concourse.bass_utils import run_bass_kernel_spmd  # noqa: E402

F32 = mybir.dt.float32
AF = mybir.ActivationFunctionType

# Model dims (hardcoded from the problem spec)
B, L, DS, DA, DC, DR, DO, HOR = 32, 16, 12, 16, 64, 256, 2, 10
NCORES, BPC = 8, 4          # batch per core
NG, FPG = 8, 8              # 8 groups of 8 frames per core (frame idx = l*4+b)
NPOS = 49                   # 7x7 patch grid
BN_EPS = 1e-5

# 'f32r' = single-pass full-rate fp32 matmul (reduced internal precision),
# 'f32'  = exact two-pass fp32. Applied via AP bitcast at matmul sites only.
MM_DT_CONV = os.environ.get("BASS_MM_DT_CONV", "f32r")
MM_DT_RNN = os.environ.get("BASS_MM_DT_RNN", "f32r")

LAST_EXEC_NS = None
LAST_RESULTS = None


def _smalls_layout():
    """Column layout of the packed per-core 'smalls' tensor [128, ncols]."""
    out = {}
    cols = 0

    def add(name, rows, width):
        nonlocal cols
        out[name] = (rows, cols, width)
        cols += width

    add("b2t", 64, 392)      # conv2 bias table (64, 49) tiled x8 frames
    add("pscale", 64, 1)     # pool+BN scale
    add("pshift", 64, 1)     # pool+BN shift
    add("xt", 12, 64)        # per-core x transposed, col = l*4+b
    add("a0t", 12, 16)
    add("a0b", 16, 1)
    add("ait", 16, 16)
    add("aib", 16, 1)
    add("ants2", 16, 256)    # an_w[:, :16].T
    add("antf", 64, 256)     # an_w[:, 16:].T
    add("anb", 128, 2)       # an_b chunks as cols
    add("bsgi", 128, 6)      # b_ih + b_hh (rz) / b_ih (n), chunk cols
    add("bsdec", 128, 24)    # same, tiled x4 batch cols
    add("bhhn", 128, 8)      # b_hh n-part, tiled x4
    add("fib", 128, 8)       # fi_b chunk cols tiled x4
    add("fnb", 2, 1)
    return out, cols


SM_LAYOUT, SM_COLS = _smalls_layout()

# conv2 shifts: center first so its start=True initializes the full PSUM rect.
SHIFTS = [(1, 1)] + [(dh, dw) for dh in range(3) for dw in range(3)
                     if (dh, dw) != (1, 1)]


def build_nc():
    nc = bacc.Bacc("TRN2", target_bir_lowering=False, debug=False,
                   num_devices=NCORES)
    mm_conv_g = mybir.dt.float32r if MM_DT_CONV == "f32r" else F32
    mm_rnn_g = mybir.dt.float32r if MM_DT_RNN == "f32r" else F32

    h_fr = nc.dram_tensor("fr", [NG, 128, 6 * 392], mm_conv_g, kind="ExternalInput")
    h_sm = nc.dram_tensor("smalls", [128, SM_COLS], mm_rnn_g, kind="ExternalInput")
    h_w1 = nc.dram_tensor("w1", [128, 6 * 576], mm_conv_g, kind="ExternalInput")
    h_w2 = nc.dram_tensor("w2", [128, 45 * 64], mm_conv_g, kind="ExternalInput")
    h_wih = nc.dram_tensor("wih", [128, 2 * 768], mm_rnn_g, kind="ExternalInput")
    h_whh = nc.dram_tensor("whh", [128, 2 * 768], mm_rnn_g, kind="ExternalInput")
    h_fi = nc.dram_tensor("fiw", [128, 2 * 256], mm_rnn_g, kind="ExternalInput")
    h_fn = nc.dram_tensor("fnw", [128, 4], mm_rnn_g, kind="ExternalInput")
    h_out = nc.dram_tensor("out", [2, 4 * HOR], F32, kind="ExternalOutput")
    debug = bool(int(os.environ.get("KERNEL_DEBUG", "0")))
    if debug:
        h_dsenc = nc.dram_tensor("d_senc", [128, 2, 64], F32,
                                 kind="ExternalOutput")
        h_dgi = nc.dram_tensor("d_gi", [128, 6, 64], F32,
                               kind="ExternalOutput")
        h_dh = nc.dram_tensor("d_h", [128, 8], F32, kind="ExternalOutput")
        h_df1 = nc.dram_tensor("d_f1", [128, 5, FPG, 9, 9], F32,
                               kind="ExternalOutput")
        h_dpg0 = nc.dram_tensor("d_pg0", [128, 48], F32,
                                kind="ExternalOutput")
        h_dhn0 = nc.dram_tensor("d_hn0", [128, 8], F32,
                                kind="ExternalOutput")
        h_dxr0 = nc.dram_tensor("d_xr0", [128, 8], F32,
                                kind="ExternalOutput")

    mm_conv = mybir.dt.float32r if MM_DT_CONV == "f32r" else F32
    mm_rnn = mybir.dt.float32r if MM_DT_RNN == "f32r" else F32
    MC = mm_conv   # dtype for conv matmul operand tensors
    MR = mm_rnn    # dtype for rnn matmul operand tensors

    mm_c2 = F32 if os.environ.get("BASS_C2_F32", "0") == "1" else mm_conv

    def mmc(out, lhsT, rhs, **kw):
        nc.tensor.matmul(out, lhsT.bitcast(mm_conv), rhs.bitcast(mm_conv),
                         skip_group_check=True, **kw)

    def mmc2(out, lhsT, rhs, **kw):
        nc.tensor.matmul(out, lhsT.bitcast(mm_c2), rhs.bitcast(mm_c2),
                         skip_group_check=True, **kw)

    def mmr(out, lhsT, rhs, **kw):
        nc.tensor.matmul(out, lhsT.bitcast(mm_rnn), rhs.bitcast(mm_rnn),
                         skip_group_check=True, **kw)

    with tile.TileContext(nc) as tc:
        from contextlib import ExitStack
        with ExitStack() as ctx:
            cpool = ctx.enter_context(tc.tile_pool(name="const", bufs=1))
            xin_pool = ctx.enter_context(tc.tile_pool(name="xin", bufs=3))
            f1_pool = ctx.enter_context(tc.tile_pool(name="f1", bufs=2))
            work = ctx.enter_context(tc.tile_pool(name="work", bufs=4))
            state = ctx.enter_context(tc.tile_pool(name="state", bufs=1))
            hpool = ctx.enter_context(tc.tile_pool(name="h", bufs=3))
            ps1 = ctx.enter_context(
                tc.tile_pool(name="ps1", bufs=4, space="PSUM"))
            ps2 = ctx.enter_context(
                tc.tile_pool(name="ps2", bufs=1, space="PSUM"))
            psr = ctx.enter_context(
                tc.tile_pool(name="psr", bufs=3, space="PSUM"))

            # ---- constants to SBUF ----
            w1 = cpool.tile([128, 6 * 576], MC, tag="w1")
            nc.sync.dma_start(w1[:], h_w1[:])
            w2 = cpool.tile([128, 45 * 64], MC, tag="w2")
            nc.sync.dma_start(w2[:], h_w2[:])
            wih = cpool.tile([128, 2 * 768], MR, tag="wih")
            nc.sync.dma_start(wih[:], h_wih[:])
            whh = cpool.tile([128, 2 * 768], MR, tag="whh")
            nc.sync.dma_start(whh[:], h_whh[:])
            fiw = cpool.tile([128, 2 * 256], MR, tag="fiw")
            nc.sync.dma_start(fiw[:], h_fi[:])
            fnw = cpool.tile([128, 4], MR, tag="fnw")
            nc.sync.dma_start(fnw[:], h_fn[:])
            sm = cpool.tile([128, SM_COLS], MR, tag="sm")
            nc.sync.dma_start(sm[:], h_sm[:])

            def sv(name):
                rows, off, width = SM_LAYOUT[name]
                return sm[0:rows, off:off + width]

            def svc(name, c0, w):  # column sub-slice of a smalls entry
                rows, off, width = SM_LAYOUT[name]
                assert c0 + w <= width
                return sm[0:rows, off + c0:off + c0 + w]

            def svf(name):  # fp32 view for ACT bias/scale and DVE operands
                return sv(name).bitcast(F32)

            def svcf(name, c0, w):
                return svc(name, c0, w).bitcast(F32)

            # ---- persistent state tiles ----
            zf1 = state.tile([128, 5 * FPG * 81], F32, tag="zf1")
            nc.gpsimd.memset(zf1[:], 0.0)
            s2 = state.tile([16, 64], MR, tag="s2")
            s_enc = state.tile([128, 2, 64], MR, tag="senc")
            GI = state.tile([128, 6, 64], F32, tag="gi")
            preds = state.tile([2, 4 * HOR], F32, tag="preds")

            # ---- state adapters: s1 = relu(a0 x); s2 = s1 + relu(ai s1) ----
            pa = psr.tile([16, 64], F32, tag="ps")
            mmr(pa[:], sv("a0t"), sv("xt"), start=True, stop=True)
            s1 = work.tile([16, 64], MR, tag="s1")
            nc.scalar.activation(s1[:], pa[:], AF.Relu, bias=svf("a0b"))
            pb = psr.tile([16, 64], F32, tag="ps")
            mmr(pb[:], sv("ait"), s1[:], start=True, stop=True)
            s1b = work.tile([16, 64], MR, tag="s1")
            nc.scalar.activation(s1b[:], pb[:], AF.Relu, bias=svf("aib"))
            nc.vector.tensor_add(s2[:], s1[:], s1b[:])

            # encoder hidden state
            h_cur = hpool.tile([128, 8], MR, tag="h")
            nc.vector.tensor_copy(h_cur[:], zf1[:, 0:8])

            def enc_step(t, h_prev):
                pg = psr.tile([128, 24], F32, tag="ps")
                for mc in range(6):
                    for kc in range(2):
                        mmr(pg[:, mc * 4:(mc + 1) * 4],
                            whh[:, kc * 768 + mc * 128:kc * 768 + (mc + 1) * 128],
                            h_prev[:, kc * 4:(kc + 1) * 4],
                            start=(kc == 0), stop=(kc == 1))
                gi_rz = GI[:, 0:4, t * 4:(t + 1) * 4]
                gi_n = GI[:, 4:6, t * 4:(t + 1) * 4]
                pre = work.tile([128, 16], F32, tag="g16")
                nc.vector.tensor_add(
                    pre[:].rearrange("p (c b) -> p c b", b=4), gi_rz,
                    pg[:, 0:16].rearrange("p (c b) -> p c b", b=4))
                rz = work.tile([128, 16], F32, tag="g16b")
                nc.scalar.activation(rz[:], pre[:], AF.Sigmoid)
                a1 = work.tile([128, 8], F32, tag="g8")
                nc.vector.tensor_add(a1[:], pg[:, 16:24], svf("bhhn"))
                a2 = work.tile([128, 8], F32, tag="g8b")
                nc.vector.tensor_mul(a2[:], rz[:, 0:8], a1[:])
                a3 = work.tile([128, 8], F32, tag="g8c")
                nc.vector.tensor_add(
                    a3[:].rearrange("p (c b) -> p c b", b=4),
                    a2[:].rearrange("p (c b) -> p c b", b=4), gi_n)
                nt = work.tile([128, 8], F32, tag="g8d")
                nc.scalar.activation(nt[:], a3[:], AF.Tanh)
                hmn = work.tile([128, 8], F32, tag="g8e")
                nc.vector.tensor_sub(hmn[:], h_prev[:], nt[:])
                zt = work.tile([128, 8], F32, tag="g8f")
                nc.vector.tensor_mul(zt[:], rz[:, 8:16], hmn[:])
                h_new = hpool.tile([128, 8], MR, tag="h")
                nc.vector.tensor_add(h_new[:], nt[:], zt[:])
                return h_new

            # ---- conv + features + GI, per group of 8 frames ----
            for g in range(NG):
                xin = xin_pool.tile([128, 6 * 392], MC, tag="xin")
                nc.sync.dma_start(xin[:], h_fr[g])

                # f1 is halo-padded to 9x9 per frame; halo + K-pad rows are
                # zeroed so every conv2 shift reads a full valid 7x7 window.
                f1 = f1_pool.tile([128, 5, FPG, 9, 9], MC, tag="f1")
                nc.vector.tensor_copy(
                    f1[:].rearrange("p a b c d -> p (a b c d)"), zf1[:])

                for m in range(5):
                    msz = 128 if m < 4 else 64
                    p1 = ps1.tile([msz, 392], F32, tag="c1")
                    for k in range(6):
                        mmc(p1[:],
                            w1[:, k * 576 + m * 128:k * 576 + m * 128 + msz],
                            xin[:, k * 392:(k + 1) * 392],
                            start=(k == 0), stop=(k == 5))
                    nc.vector.tensor_copy(
                        f1[0:msz, m, :, 1:8, 1:8],
                        p1[:].rearrange("p (f a b) -> p f a b", a=7, b=7))

                p2 = ps2.tile([64, FPG * 49], F32, tag="c2")
                for si, (dh, dw) in enumerate(SHIFTS):
                    s = dh * 3 + dw
                    for k in range(5):
                        rhs = f1[:, k, :, dh:dh + 7, dw:dw + 7]
                        mmc2(p2[:],
                             w2[:, (s * 5 + k) * 64:(s * 5 + k + 1) * 64],
                             rhs, start=(si == 0 and k == 0),
                             stop=(si == len(SHIFTS) - 1 and k == 4))

                # epilogue: relu(conv2 + B2) -> mean -> BN affine
                t0 = work.tile([64, FPG * 49], F32, tag="ep")
                nc.vector.tensor_add(t0[:], p2[:], svf("b2t"))
                t1 = work.tile([64, FPG * 49], F32, tag="ep2")
                nc.vector.tensor_scalar_max(t1[:], t0[:], 0.0)
                red = work.tile([64, FPG], F32, tag="red")
                nc.vector.tensor_reduce(
                    red[:], t1[:].rearrange("p (f s) -> p f s", s=49),
                    axis=mybir.AxisListType.X, op=mybir.AluOpType.add)
                feats = work.tile([64, FPG], MR, tag="feats")
                nc.scalar.activation(feats[:], red[:], AF.Identity,
                                     bias=svf("pshift"), scale=svf("pscale"))

                # an: relu(an_w [s2; feats] + an_b), K split 64(feats)+16(s2)
                gcol = slice(g * FPG, (g + 1) * FPG)
                for mc in range(2):
                    pan = psr.tile([128, FPG], F32, tag="ps")
                    mmr(pan[:], svc("antf", mc * 128, 128), feats[:],
                        start=True, stop=False)
                    mmr(pan[:], svc("ants2", mc * 128, 128), s2[:, gcol],
                        start=False, stop=True)
                    nc.scalar.activation(s_enc[:, mc, gcol], pan[:], AF.Relu,
                                         bias=svcf("anb", mc, 1))

                # GI = w_ih @ s_enc + (b_ih + b_hh fold) for these 8 cols
                for mc in range(6):
                    pgi = psr.tile([128, FPG], F32, tag="ps")
                    for kc in range(2):
                        mmr(pgi[:],
                            wih[:, kc * 768 + mc * 128:kc * 768 + (mc + 1) * 128],
                            s_enc[:, kc, gcol],
                            start=(kc == 0), stop=(kc == 1))
                    nc.scalar.activation(GI[:, mc, gcol], pgi[:], AF.Identity,
                                         bias=svcf("bsgi", mc, 1))

                if debug and g == 0:
                    nc.sync.dma_start(h_df1[:], f1[:])

                # encoder steps that become ready after this group
                h_cur = enc_step(2 * g, h_cur)
                h_cur = enc_step(2 * g + 1, h_cur)

            if debug:
                nc.sync.dma_start(h_dsenc[:], s_enc[:])
                nc.sync.dma_start(h_dgi[:], GI[:])
                nc.sync.dma_start(h_dh[:], h_cur[:])

            # ---- decoder ----
            xi, hh = h_cur, h_cur
            for t in range(HOR):
                pg = psr.tile([128, 48], F32, tag="ps")
                # each PSUM accumulation group must complete before the next
                # one starts (interleaved groups break accumulation)
                for mc in range(6):
                    for kc in range(2):
                        mmr(pg[:, mc * 4:(mc + 1) * 4],
                            wih[:, kc * 768 + mc * 128:
                                kc * 768 + (mc + 1) * 128],
                            xi[:, kc * 4:(kc + 1) * 4],
                            start=(kc == 0), stop=(kc == 1))
                    for kc in range(2):
                        mmr(pg[:, 24 + mc * 4:24 + (mc + 1) * 4],
                            whh[:, kc * 768 + mc * 128:
                                kc * 768 + (mc + 1) * 128],
                            hh[:, kc * 4:(kc + 1) * 4],
                            start=(kc == 0), stop=(kc == 1))
                if debug and t == 0:
                    pgc = work.tile([128, 48], F32, tag="dbgpg")
                    nc.vector.tensor_copy(pgc[:], pg[:])
                    nc.sync.dma_start(h_dpg0[:], pgc[:])
                gisb = work.tile([128, 24], F32, tag="g24")
                nc.vector.tensor_add(gisb[:], pg[:, 0:24], svf("bsdec"))
                pre = work.tile([128, 16], F32, tag="g16")
                nc.vector.tensor_add(pre[:], gisb[:, 0:16], pg[:, 24:40])
                rz = work.tile([128, 16], F32, tag="g16b")
                nc.scalar.activation(rz[:], pre[:], AF.Sigmoid)
                a1 = work.tile([128, 8], F32, tag="g8")
                nc.vector.tensor_add(a1[:], pg[:, 40:48], svf("bhhn"))
                a2 = work.tile([128, 8], F32, tag="g8b")
                nc.vector.tensor_mul(a2[:], rz[:, 0:8], a1[:])
                a3 = work.tile([128, 8], F32, tag="g8c")
                nc.vector.tensor_add(a3[:], a2[:], gisb[:, 16:24])
                nt = work.tile([128, 8], F32, tag="g8d")
                nc.scalar.activation(nt[:], a3[:], AF.Tanh)
                hmn = work.tile([128, 8], F32, tag="g8e")
                nc.vector.tensor_sub(hmn[:], hh[:], nt[:])
                zt = work.tile([128, 8], F32, tag="g8f")
                nc.vector.tensor_mul(zt[:], rz[:, 8:16], hmn[:])
                hn = hpool.tile([128, 8], MR, tag="h")
                nc.vector.tensor_add(hn[:], nt[:], zt[:])

                pfi = psr.tile([128, 8], F32, tag="ps")
                for mc2 in range(2):
                    for kc2 in range(2):
                        mmr(pfi[:, mc2 * 4:(mc2 + 1) * 4],
                            fiw[:, kc2 * 256 + mc2 * 128:
                                kc2 * 256 + (mc2 + 1) * 128],
                            hn[:, kc2 * 4:(kc2 + 1) * 4],
                            start=(kc2 == 0), stop=(kc2 == 1))
                u1 = work.tile([128, 8], F32, tag="g8")
                nc.vector.tensor_add(u1[:], pfi[:], svf("fib"))
                u2 = work.tile([128, 8], F32, tag="g8b")
                nc.vector.tensor_scalar_max(u2[:], u1[:], 0.0)
                xr = hpool.tile([128, 8], MR, tag="xr")
                nc.vector.tensor_add(xr[:], hn[:], u2[:])

                pfn = psr.tile([2, 4], F32, tag="ps")
                for kc in range(2):
                    mmr(pfn[:], fnw[:, kc * 2:(kc + 1) * 2],
                        xr[:, kc * 4:(kc + 1) * 4],
                        start=(kc == 0), stop=(kc == 1))
                nc.scalar.activation(preds[:, t * 4:(t + 1) * 4], pfn[:],
                                     AF.Tanh, bias=svf("fnb"))
                if debug and t == 0:
                    nc.sync.dma_start(h_dhn0[:], hn[:])
                    nc.sync.dma_start(h_dxr0[:], xr[:])
                xi, hh = xr, hn

            nc.sync.dma_start(h_out[:], preds[:])

    nc.finalize()
    return nc


# ---------------- host-side data prep ----------------

def _prep_frames(frames):
    """frames (32,16,3,112,112) -> per-core [NG, 128, 2352] patch-T layout."""
    out = np.empty((NCORES, NG, 128, 6 * 392), np.float32)
    fr = np.ascontiguousarray(frames, np.float32)
    for c in range(NCORES):
        fb = fr[c * BPC:(c + 1) * BPC]  # (4, 16, 3, 112, 112)
        a = fb.reshape(BPC, L, 3, 7, 16, 7, 16)
        # -> [l, b, ch, kh, kw, ph, pw]
        a = a.transpose(1, 0, 2, 4, 6, 3, 5)
        a = a.reshape(L, BPC, 768, 49)
        a = a.reshape(NG, 2, BPC, 6, 128, 49)
        # -> [g, k, p, li, b, s]
        a = a.transpose(0, 3, 4, 1, 2, 5)
        a = a.reshape(NG, 6, 128, 392)
        a = a.transpose(0, 2, 1, 3)  # [g, p, k, 392]
        out[c] = a.reshape(NG, 128, 6 * 392)
    return out


def _prep_weights(iv):
    w = {}
    W1f = iv["cnn_w"].reshape(576, 768).astype(np.float32)
    w["w1"] = np.ascontiguousarray(
        W1f.T.reshape(6, 128, 576).transpose(1, 0, 2).reshape(128, 6 * 576))

    w2h = np.zeros((9, 5, 128, 64), np.float32)
    for dh in range(3):
        for dw in range(3):
            s = dh * 3 + dw
            T = iv["cnn1_w"][:, :, dh, dw].T.astype(np.float32)  # (576, 64)
            Tp = np.zeros((640, 64), np.float32)
            Tp[:576] = T
            w2h[s] = Tp.reshape(5, 128, 64)
    w["w2"] = np.ascontiguousarray(
        w2h.transpose(2, 0, 1, 3).reshape(128, 45 * 64))

    for name, key in (("wih", "w_ih"), ("whh", "w_hh")):
        T = iv[key].T.astype(np.float32)  # (256, 768)
        w[name] = np.ascontiguousarray(
            T.reshape(2, 128, 768).transpose(1, 0, 2).reshape(128, 1536))
    T = iv["fi_w"].T.astype(np.float32)  # (256, 256)
    w["fiw"] = np.ascontiguousarray(
        T.reshape(2, 128, 256).transpose(1, 0, 2).reshape(128, 512))
    T = iv["fn_w"].T.astype(np.float32)  # (256, 2)
    w["fnw"] = np.ascontiguousarray(
        T.reshape(2, 128, 2).transpose(1, 0, 2).reshape(128, 4))
    return w


def _prep_smalls(iv, x, core):
    sm = np.zeros((128, SM_COLS), np.float32)

    def put(name, arr):
        rows, off, width = SM_LAYOUT[name]
        a = np.asarray(arr, np.float32).reshape(rows, width)
        sm[0:rows, off:off + width] = a

    # conv2 position-dependent bias fold (conv1 bias + cnn1_b)
    M = np.einsum("oiab,i->oab", iv["cnn1_w"], iv["cnn_b"]).astype(np.float32)
    B2 = np.zeros((64, 7, 7), np.float32)
    for ph in range(7):
        for pw in range(7):
            acc = iv["cnn1_b"].astype(np.float32).copy()
            for dh in range(3):
                for dw in range(3):
                    if 0 <= ph + dh - 1 <= 6 and 0 <= pw + dw - 1 <= 6:
                        acc = acc + M[:, dh, dw]
            B2[:, ph, pw] = acc
    put("b2t", np.tile(B2.reshape(64, 49), (1, FPG)))

    inv = iv["bn_g"] / np.sqrt(iv["bn_v"] + BN_EPS)
    put("pscale", (inv / 49.0)[:, None])
    put("pshift", (iv["bn_b"] - iv["bn_m"] * inv)[:, None])

    xb = x[core * BPC:(core + 1) * BPC]  # (4, 16, 12)
    put("xt", xb.transpose(2, 1, 0).reshape(12, 64))

    put("a0t", iv["a0_w"].T)
    put("a0b", iv["a0_b"][:, None])
    put("ait", iv["ai_w"].T)
    put("aib", iv["ai_b"][:, None])
    put("ants2", iv["an_w"][:, 0:16].T)
    put("antf", iv["an_w"][:, 16:80].T)
    put("anb", iv["an_b"].reshape(2, 128).T)

    bs = (iv["b_ih"] + iv["b_hh"]).astype(np.float32)
    bs[512:] = iv["b_ih"][512:]
    put("bsgi", bs.reshape(6, 128).T)
    put("bsdec", np.repeat(bs.reshape(6, 128).T, 4, axis=1))
    put("bhhn", np.repeat(iv["b_hh"][512:].reshape(2, 128).T, 4, axis=1))
    put("fib", np.repeat(iv["fi_b"].reshape(2, 128).T, 4, axis=1))
    put("fnb", iv["fn_b"][:, None])
    return sm


def make_in_maps(inputs):
    iv = {k: np.asarray(v, np.float32) for k, v in inputs.items()}
    frames = iv["frames"]
    x = iv["x"]
    fr_all = _prep_frames(frames)
    w = _prep_weights(iv)
    in_maps = []
    for c in range(NCORES):
        m = {"fr": np.ascontiguousarray(fr_all[c]),
             "smalls": _prep_smalls(iv, x, c)}
        m.update(w)
        in_maps.append(m)
    return in_maps


_NC_CACHE = None


def get_nc():
    global _NC_CACHE
    if _NC_CACHE is None:
        _NC_CACHE = build_nc()
    return _NC_CACHE


def _install_ntff_hook():
    """The agent image's antenv lacks axon_hooks; synthesize it so
    run_bass_kernel_spmd(trace=True) can capture NTFF profiles."""
    try:
        from antenv.axon_hooks import get_axon_ntff_profile_hook  # noqa: F401
        return True
    except ImportError:
        pass
    try:
        import types
        import antenv
        if "/root/.axon_site" not in sys.path:
            sys.path.insert(0, "/root/.axon_site")
        from trn_agent_boot.trn_boot import _ntff_profile_via_ctypes
        hook = _ntff_profile_via_ctypes("/opt/axon/libaxon_pjrt.so")
        mod = types.ModuleType("antenv.axon_hooks")
        mod.get_axon_ntff_profile_hook = lambda: hook
        mod.set_axon_ntff_profile_hook = lambda h: None
        sys.modules["antenv.axon_hooks"] = mod
        antenv.axon_hooks = mod
        return hook is not None
    except Exception as e:  # pragma: no cover - profiling is best-effort
        print(f"ntff hook install failed: {e}")
        return False


def kernel(**inputs):
    global LAST_EXEC_NS, LAST_RESULTS
    nc = get_nc()
    in_maps = make_in_maps(inputs)
    trace = bool(int(os.environ.get("KERNEL_TRACE", "0")))
    if trace:
        trace = _install_ntff_hook()
    res = run_bass_kernel_spmd(nc, in_maps, core_ids=list(range(NCORES)),
                               trace=trace)
    LAST_RESULTS = res
    LAST_EXEC_NS = res.exec_time_ns
    outs = []
    for c in range(NCORES):
        o = res.results[c]["out"]  # (2, 40)
        outs.append(o.reshape(2, HOR, BPC).transpose(1, 2, 0)[:, :, None, :])
    return np.concatenate(outs, axis=1).astype(np.float32)


if __name__ == "__main__":
    nc = get_nc()
    print("built ok; instructions:",
          sum(len(bb.instructions) for bb in nc.main_func.blocks))


# revision 21
# speedup vs baseline: 2.5139x; 1.5442x over previous
"""CRNN Trainium2 kernel: patchify-conv -> 3x3 conv -> pool -> GRU encoder ->
autoregressive GRU decoder. Pure data-parallel over batch (32 -> 8 cores x 4).

Host side: numpy layout transforms (patch-transpose of frames, weight
re-layouts, BN/bias folding). Device side: one Bass/Tile SPMD program run on 8
NeuronCores via run_bass_kernel_spmd.
"""

import os
import sys

for _p in ("/opt/trn_rl_repo", "/root/.axon_site/_ro/trn_rl_repo"):
    if os.path.isdir(_p) and _p not in sys.path:
        sys.path.insert(0, _p)

import numpy as np

import concourse.bass as bass  # noqa: E402
import concourse.mybir as mybir  # noqa: E402
import concourse.tile as tile  # noqa: E402
from concourse import bacc  # noqa: E402
from concourse.bass_utils import run_bass_kernel_spmd  # noqa: E402

F32 = mybir.dt.float32
AF = mybir.ActivationFunctionType

# Model dims (hardcoded from the problem spec)
B, L, DS, DA, DC, DR, DO, HOR = 32, 16, 12, 16, 64, 256, 2, 10
NCORES, BPC = 8, 4          # batch per core
NG, FPG = 8, 8              # 8 groups of 8 frames per core (frame idx = l*4+b)
NPOS = 49                   # 7x7 patch grid
BN_EPS = 1e-5

# 'f32r' = single-pass full-rate fp32 matmul (reduced internal precision),
# 'f32'  = exact two-pass fp32. Applied via AP bitcast at matmul sites only.
MM_DT_CONV = os.environ.get("BASS_MM_DT_CONV", "f32r")
MM_DT_RNN = os.environ.get("BASS_MM_DT_RNN", "f32r")

LAST_EXEC_NS = None
LAST_RESULTS = None


def _smalls_layout():
    """Column layout of the packed per-core 'smalls' tensor [128, ncols]."""
    out = {}
    cols = 0

    def add(name, rows, width):
        nonlocal cols
        out[name] = (rows, cols, width)
        cols += width

    add("b2t", 64, 392)      # conv2 bias table (64, 49) tiled x8 frames
    add("pscale", 64, 1)     # pool+BN scale
    add("pshift", 64, 1)     # pool+BN shift
    add("xt", 12, 64)        # per-core x transposed, col = l*4+b
    add("a0t", 12, 16)
    add("a0b", 16, 1)
    add("ait", 16, 16)
    add("aib", 16, 1)
    add("ants2", 16, 256)    # an_w[:, :16].T
    add("antf", 64, 256)     # an_w[:, 16:].T
    add("anb", 128, 2)       # an_b chunks as cols
    add("bsgi", 128, 6)      # b_ih + b_hh (rz) / b_ih (n), chunk cols
    add("bsdec", 128, 24)    # same, tiled x4 batch cols
    add("bhhn", 128, 8)      # b_hh n-part, tiled x4
    add("fib", 128, 8)       # fi_b chunk cols tiled x4
    add("fnb", 2, 1)
    return out, cols


SM_LAYOUT, SM_COLS = _smalls_layout()

# conv2 shifts: center first so its start=True initializes the full PSUM rect.
SHIFTS = [(1, 1)] + [(dh, dw) for dh in range(3) for dw in range(3)
                     if (dh, dw) != (1, 1)]


def build_nc():
    nc = bacc.Bacc("TRN2", target_bir_lowering=False, debug=False,
                   num_devices=NCORES)
    mm_conv_g = mybir.dt.float32r if MM_DT_CONV == "f32r" else F32
    mm_rnn_g = mybir.dt.float32r if MM_DT_RNN == "f32r" else F32

    h_fr = nc.dram_tensor("fr", [NG, 128, 6 * 392], mm_conv_g, kind="ExternalInput")
    h_sm = nc.dram_tensor("smalls", [128, SM_COLS], mm_rnn_g, kind="ExternalInput")
    h_w1 = nc.dram_tensor("w1", [128, 6 * 576], mm_conv_g, kind="ExternalInput")
    h_w2 = nc.dram_tensor("w2", [128, 45 * 64], mm_conv_g, kind="ExternalInput")
    h_wih = nc.dram_tensor("wih", [128, 2 * 768], mm_rnn_g, kind="ExternalInput")
    h_whh = nc.dram_tensor("whh", [128, 2 * 768], mm_rnn_g, kind="ExternalInput")
    h_fi = nc.dram_tensor("fiw", [128, 2 * 256], mm_rnn_g, kind="ExternalInput")
    h_fn = nc.dram_tensor("fnw", [128, 4], mm_rnn_g, kind="ExternalInput")
    h_out = nc.dram_tensor("out", [2, 4 * HOR], F32, kind="ExternalOutput")
    debug = bool(int(os.environ.get("KERNEL_DEBUG", "0")))
    if debug:
        h_dsenc = nc.dram_tensor("d_senc", [128, 2, 64], F32,
                                 kind="ExternalOutput")
        h_dgi = nc.dram_tensor("d_gi", [128, 6, 64], F32,
                               kind="ExternalOutput")
        h_dh = nc.dram_tensor("d_h", [128, 8], F32, kind="ExternalOutput")
        h_df1 = nc.dram_tensor("d_f1", [128, 5, FPG, 9, 9], F32,
                               kind="ExternalOutput")
        h_dpg0 = nc.dram_tensor("d_pg0", [128, 48], F32,
                                kind="ExternalOutput")
        h_dhn0 = nc.dram_tensor("d_hn0", [128, 8], F32,
                                kind="ExternalOutput")
        h_dxr0 = nc.dram_tensor("d_xr0", [128, 8], F32,
                                kind="ExternalOutput")

    mm_conv = mybir.dt.float32r if MM_DT_CONV == "f32r" else F32
    mm_rnn = mybir.dt.float32r if MM_DT_RNN == "f32r" else F32
    MC = mm_conv   # dtype for conv matmul operand tensors
    MR = mm_rnn    # dtype for rnn matmul operand tensors

    mm_c2 = F32 if os.environ.get("BASS_C2_F32", "0") == "1" else mm_conv

    def mmc(out, lhsT, rhs, **kw):
        nc.tensor.matmul(out, lhsT.bitcast(mm_conv), rhs.bitcast(mm_conv),
                         skip_group_check=True, **kw)

    def mmc2(out, lhsT, rhs, **kw):
        nc.tensor.matmul(out, lhsT.bitcast(mm_c2), rhs.bitcast(mm_c2),
                         skip_group_check=True, **kw)

    def mmr(out, lhsT, rhs, **kw):
        nc.tensor.matmul(out, lhsT.bitcast(mm_rnn), rhs.bitcast(mm_rnn),
                         skip_group_check=True, **kw)

    with tile.TileContext(nc) as tc:
        from contextlib import ExitStack
        with ExitStack() as ctx:
            cpool = ctx.enter_context(tc.tile_pool(name="const", bufs=1))
            xin_pool = ctx.enter_context(tc.tile_pool(name="xin", bufs=3))
            f1_pool = ctx.enter_context(tc.tile_pool(name="f1", bufs=2))
            work = ctx.enter_context(tc.tile_pool(name="work", bufs=4))
            state = ctx.enter_context(tc.tile_pool(name="state", bufs=1))
            hpool = ctx.enter_context(tc.tile_pool(name="h", bufs=3))
            ps1 = ctx.enter_context(
                tc.tile_pool(name="ps1", bufs=3, space="PSUM"))
            ps2 = ctx.enter_context(
                tc.tile_pool(name="ps2", bufs=1, space="PSUM"))
            psr = ctx.enter_context(
                tc.tile_pool(name="psr", bufs=3, space="PSUM"))

            # ---- constants to SBUF ----
            w1 = cpool.tile([128, 6 * 576], MC, tag="w1")
            nc.sync.dma_start(w1[:], h_w1[:])
            w2 = cpool.tile([128, 45 * 64], MC, tag="w2")
            nc.sync.dma_start(w2[:], h_w2[:])
            wih = cpool.tile([128, 2 * 768], MR, tag="wih")
            nc.sync.dma_start(wih[:], h_wih[:])
            whh = cpool.tile([128, 2 * 768], MR, tag="whh")
            nc.sync.dma_start(whh[:], h_whh[:])
            fiw = cpool.tile([128, 2 * 256], MR, tag="fiw")
            nc.sync.dma_start(fiw[:], h_fi[:])
            fnw = cpool.tile([128, 4], MR, tag="fnw")
            nc.sync.dma_start(fnw[:], h_fn[:])
            sm = cpool.tile([128, SM_COLS], MR, tag="sm")
            nc.sync.dma_start(sm[:], h_sm[:])

            def sv(name):
                rows, off, width = SM_LAYOUT[name]
                return sm[0:rows, off:off + width]

            def svc(name, c0, w):  # column sub-slice of a smalls entry
                rows, off, width = SM_LAYOUT[name]
                assert c0 + w <= width
                return sm[0:rows, off + c0:off + c0 + w]

            def svf(name):  # fp32 view for ACT bias/scale and DVE operands
                return sv(name).bitcast(F32)

            def svcf(name, c0, w):
                return svc(name, c0, w).bitcast(F32)

            # ---- persistent state tiles ----
            # conv2 gutter-flat layout: 8x8 cells per frame (row 0 + col 0 of
            # each 8-cell row are zero gutters), LEAD/TAIL pads absorb shift
            # window spill. All 9 conv2 taps become contiguous 2D windows.
            LEAD, FB, FW = 16, 64, 16 + 8 * 64 + 16  # 544 per ic-chunk
            zf1 = state.tile([128, 5 * FW], F32, tag="zf1")
            nc.gpsimd.memset(zf1[:], 0.0)
            s2 = state.tile([16, 64], MR, tag="s2")
            s_enc = state.tile([128, 2, 64], MR, tag="senc")
            GI = state.tile([128, 6, 64], F32, tag="gi")
            preds = state.tile([2, 4 * HOR], F32, tag="preds")

            # ---- state adapters: s1 = relu(a0 x); s2 = s1 + relu(ai s1) ----
            pa = psr.tile([16, 64], F32, tag="ps")
            mmr(pa[:], sv("a0t"), sv("xt"), start=True, stop=True)
            s1 = work.tile([16, 64], MR, tag="s1")
            nc.scalar.activation(s1[:], pa[:], AF.Relu, bias=svf("a0b"))
            pb = psr.tile([16, 64], F32, tag="ps")
            mmr(pb[:], sv("ait"), s1[:], start=True, stop=True)
            s1b = work.tile([16, 64], MR, tag="s1")
            nc.scalar.activation(s1b[:], pb[:], AF.Relu, bias=svf("aib"))
            nc.vector.tensor_add(s2[:], s1[:], s1b[:])

            # encoder hidden state
            h_cur = hpool.tile([128, 8], MR, tag="h")
            nc.vector.tensor_copy(h_cur[:], zf1[:, 0:8])

            def enc_step(t, h_prev):
                pg = psr.tile([128, 24], F32, tag="ps")
                for mc in range(6):
                    for kc in range(2):
                        mmr(pg[:, mc * 4:(mc + 1) * 4],
                            whh[:, kc * 768 + mc * 128:kc * 768 + (mc + 1) * 128],
                            h_prev[:, kc * 4:(kc + 1) * 4],
                            start=(kc == 0), stop=(kc == 1))
                gi_rz = GI[:, 0:4, t * 4:(t + 1) * 4]
                gi_n = GI[:, 4:6, t * 4:(t + 1) * 4]
                pre = work.tile([128, 16], F32, tag="g16")
                nc.vector.tensor_add(
                    pre[:].rearrange("p (c b) -> p c b", b=4), gi_rz,
                    pg[:, 0:16].rearrange("p (c b) -> p c b", b=4))
                rz = work.tile([128, 16], F32, tag="g16b")
                nc.scalar.activation(rz[:], pre[:], AF.Sigmoid)
                a1 = work.tile([128, 8], F32, tag="g8")
                nc.vector.tensor_add(a1[:], pg[:, 16:24], svf("bhhn"))
                a2 = work.tile([128, 8], F32, tag="g8b")
                nc.vector.tensor_mul(a2[:], rz[:, 0:8], a1[:])
                a3 = work.tile([128, 8], F32, tag="g8c")
                nc.vector.tensor_add(
                    a3[:].rearrange("p (c b) -> p c b", b=4),
                    a2[:].rearrange("p (c b) -> p c b", b=4), gi_n)
                nt = work.tile([128, 8], F32, tag="g8d")
                nc.scalar.activation(nt[:], a3[:], AF.Tanh)
                hmn = work.tile([128, 8], F32, tag="g8e")
                nc.vector.tensor_sub(hmn[:], h_prev[:], nt[:])
                zt = work.tile([128, 8], F32, tag="g8f")
                nc.vector.tensor_mul(zt[:], rz[:, 8:16], hmn[:])
                h_new = hpool.tile([128, 8], MR, tag="h")
                nc.vector.tensor_add(h_new[:], nt[:], zt[:])
                return h_new

            # ---- conv + features + GI, per group of 8 frames ----
            for g in range(NG):
                xin = xin_pool.tile([128, 6 * 392], MC, tag="xin")
                nc.sync.dma_start(xin[:], h_fr[g])

                # f1 in gutter-flat layout: data (f, ph, pw) at col
                # LEAD + f*64 + (ph+1)*8 + (pw+1); gutters stay zero.
                f1 = f1_pool.tile([128, 5, FW], MC, tag="f1")
                nc.vector.tensor_copy(
                    f1[:].rearrange("p a b -> p (a b)"), zf1[:])

                for m in range(5):
                    msz = 128 if m < 4 else 64
                    p1 = ps1.tile([msz, 392], F32, tag="c1")
                    for k in range(6):
                        mmc(p1[:],
                            w1[:, k * 576 + m * 128:k * 576 + m * 128 + msz],
                            xin[:, k * 392:(k + 1) * 392],
                            start=(k == 0), stop=(k == 5))
                    dst = f1[0:msz, m, LEAD:LEAD + 8 * FB].rearrange(
                        "p (f a b) -> p f a b", a=8, b=8)[:, :, 1:8, 1:8]
                    nc.vector.tensor_copy(
                        dst, p1[:].rearrange("p (f a b) -> p f a b", a=7, b=7))

                # conv2: 9 taps x 5 ic-chunks x 2 frame-halves, each a plain
                # contiguous window matmul (N=256, fp32r full rate)
                p2a = ps2.tile([64, 256], F32, tag="c2a", name=f"p2a_{g}")
                p2b = ps2.tile([64, 256], F32, tag="c2b", name=f"p2b_{g}")
                p2h = [p2a, p2b]
                for si, (dh, dw) in enumerate(SHIFTS):
                    s = dh * 3 + dw
                    delta = (dh - 1) * 8 + (dw - 1)
                    for k in range(5):
                        for h in (0, 1):
                            a = LEAD + h * 256 + delta
                            mmc2(p2h[h][:],
                                 w2[:, (s * 5 + k) * 64:(s * 5 + k + 1) * 64],
                                 f1[:, k, a:a + 256],
                                 start=(si == 0 and k == 0),
                                 stop=(si == len(SHIFTS) - 1 and k == 4))

                # epilogue: relu(conv2 + B2) -> mean -> BN affine (per half)
                red = work.tile([64, FPG], F32, tag="red")
                for h in (0, 1):
                    pv = p2h[h][:].rearrange(
                        "p (f a b) -> p f a b", a=8, b=8)[:, :, 1:8, 1:8]
                    b2v = svcf("b2t", h * 196, 196).rearrange(
                        "p (f a b) -> p f a b", a=7, b=7)
                    t0 = work.tile([64, 4, 7, 7], F32, tag="ep")
                    nc.vector.tensor_add(t0[:], pv, b2v)
                    t1 = work.tile([64, 4, 7, 7], F32, tag="ep2")
                    nc.vector.tensor_scalar_max(t1[:], t0[:], 0.0)
                    nc.vector.tensor_reduce(
                        red[:, h * 4:(h + 1) * 4], t1[:],
                        axis=mybir.AxisListType.XY, op=mybir.AluOpType.add)
                feats = work.tile([64, FPG], MR, tag="feats")
                nc.scalar.activation(feats[:], red[:], AF.Identity,
                                     bias=svf("pshift"), scale=svf("pscale"))

                # an: relu(an_w [s2; feats] + an_b), K split 64(feats)+16(s2)
                gcol = slice(g * FPG, (g + 1) * FPG)
                for mc in range(2):
                    pan = psr.tile([128, FPG], F32, tag="ps")
                    mmr(pan[:], svc("antf", mc * 128, 128), feats[:],
                        start=True, stop=False)
                    mmr(pan[:], svc("ants2", mc * 128, 128), s2[:, gcol],
                        start=False, stop=True)
                    nc.scalar.activation(s_enc[:, mc, gcol], pan[:], AF.Relu,
                                         bias=svcf("anb", mc, 1))

                # GI = w_ih @ s_enc + (b_ih + b_hh fold) for these 8 cols
                for mc in range(6):
                    pgi = psr.tile([128, FPG], F32, tag="ps")
                    for kc in range(2):
                        mmr(pgi[:],
                            wih[:, kc * 768 + mc * 128:kc * 768 + (mc + 1) * 128],
                            s_enc[:, kc, gcol],
                            start=(kc == 0), stop=(kc == 1))
                    nc.scalar.activation(GI[:, mc, gcol], pgi[:], AF.Identity,
                                         bias=svcf("bsgi", mc, 1))

                if debug and g == 0:
                    nc.sync.dma_start(h_df1[:], f1[:])

                # encoder steps that become ready after this group
                h_cur = enc_step(2 * g, h_cur)
                h_cur = enc_step(2 * g + 1, h_cur)

            if debug:
                nc.sync.dma_start(h_dsenc[:], s_enc[:])
                nc.sync.dma_start(h_dgi[:], GI[:])
                nc.sync.dma_start(h_dh[:], h_cur[:])

            # ---- decoder ----
            xi, hh = h_cur, h_cur
            for t in range(HOR):
                pg = psr.tile([128, 48], F32, tag="ps")
                # each PSUM accumulation group must complete before the next
                # one starts (interleaved groups break accumulation)
                for mc in range(6):
                    for kc in range(2):
                        mmr(pg[:, mc * 4:(mc + 1) * 4],
                            wih[:, kc * 768 + mc * 128:
                                kc * 768 + (mc + 1) * 128],
                            xi[:, kc * 4:(kc + 1) * 4],
                            start=(kc == 0), stop=(kc == 1))
                    for kc in range(2):
                        mmr(pg[:, 24 + mc * 4:24 + (mc + 1) * 4],
                            whh[:, kc * 768 + mc * 128:
                                kc * 768 + (mc + 1) * 128],
                            hh[:, kc * 4:(kc + 1) * 4],
                            start=(kc == 0), stop=(kc == 1))
                if debug and t == 0:
                    pgc = work.tile([128, 48], F32, tag="dbgpg")
                    nc.vector.tensor_copy(pgc[:], pg[:])
                    nc.sync.dma_start(h_dpg0[:], pgc[:])
                gisb = work.tile([128, 24], F32, tag="g24")
                nc.vector.tensor_add(gisb[:], pg[:, 0:24], svf("bsdec"))
                pre = work.tile([128, 16], F32, tag="g16")
                nc.vector.tensor_add(pre[:], gisb[:, 0:16], pg[:, 24:40])
                rz = work.tile([128, 16], F32, tag="g16b")
                nc.scalar.activation(rz[:], pre[:], AF.Sigmoid)
                a1 = work.tile([128, 8], F32, tag="g8")
                nc.vector.tensor_add(a1[:], pg[:, 40:48], svf("bhhn"))
                a2 = work.tile([128, 8], F32, tag="g8b")
                nc.vector.tensor_mul(a2[:], rz[:, 0:8], a1[:])
                a3 = work.tile([128, 8], F32, tag="g8c")
                nc.vector.tensor_add(a3[:], a2[:], gisb[:, 16:24])
                nt = work.tile([128, 8], F32, tag="g8d")
                nc.scalar.activation(nt[:], a3[:], AF.Tanh)
                hmn = work.tile([128, 8], F32, tag="g8e")
                nc.vector.tensor_sub(hmn[:], hh[:], nt[:])
                zt = work.tile([128, 8], F32, tag="g8f")
                nc.vector.tensor_mul(zt[:], rz[:, 8:16], hmn[:])
                hn = hpool.tile([128, 8], MR, tag="h")
                nc.vector.tensor_add(hn[:], nt[:], zt[:])

                pfi = psr.tile([128, 8], F32, tag="ps")
                for mc2 in range(2):
                    for kc2 in range(2):
                        mmr(pfi[:, mc2 * 4:(mc2 + 1) * 4],
                            fiw[:, kc2 * 256 + mc2 * 128:
                                kc2 * 256 + (mc2 + 1) * 128],
                            hn[:, kc2 * 4:(kc2 + 1) * 4],
                            start=(kc2 == 0), stop=(kc2 == 1))
                u1 = work.tile([128, 8], F32, tag="g8")
                nc.vector.tensor_add(u1[:], pfi[:], svf("fib"))
                u2 = work.tile([128, 8], F32, tag="g8b")
                nc.vector.tensor_scalar_max(u2[:], u1[:], 0.0)
                xr = hpool.tile([128, 8], MR, tag="xr")
                nc.vector.tensor_add(xr[:], hn[:], u2[:])

                pfn = psr.tile([2, 4], F32, tag="ps")
                for kc in range(2):
                    mmr(pfn[:], fnw[:, kc * 2:(kc + 1) * 2],
                        xr[:, kc * 4:(kc + 1) * 4],
                        start=(kc == 0), stop=(kc == 1))
                nc.scalar.activation(preds[:, t * 4:(t + 1) * 4], pfn[:],
                                     AF.Tanh, bias=svf("fnb"))
                if debug and t == 0:
                    nc.sync.dma_start(h_dhn0[:], hn[:])
                    nc.sync.dma_start(h_dxr0[:], xr[:])
                xi, hh = xr, hn

            nc.sync.dma_start(h_out[:], preds[:])

    nc.finalize()
    return nc


# ---------------- host-side data prep ----------------

def _prep_frames(frames):
    """frames (32,16,3,112,112) -> per-core [NG, 128, 2352] patch-T layout."""
    out = np.empty((NCORES, NG, 128, 6 * 392), np.float32)
    fr = np.ascontiguousarray(frames, np.float32)
    for c in range(NCORES):
        fb = fr[c * BPC:(c + 1) * BPC]  # (4, 16, 3, 112, 112)
        a = fb.reshape(BPC, L, 3, 7, 16, 7, 16)
        # -> [l, b, ch, kh, kw, ph, pw]
        a = a.transpose(1, 0, 2, 4, 6, 3, 5)
        a = a.reshape(L, BPC, 768, 49)
        a = a.reshape(NG, 2, BPC, 6, 128, 49)
        # -> [g, k, p, li, b, s]
        a = a.transpose(0, 3, 4, 1, 2, 5)
        a = a.reshape(NG, 6, 128, 392)
        a = a.transpose(0, 2, 1, 3)  # [g, p, k, 392]
        out[c] = a.reshape(NG, 128, 6 * 392)
    return out


def _prep_weights(iv):
    w = {}
    W1f = iv["cnn_w"].reshape(576, 768).astype(np.float32)
    w["w1"] = np.ascontiguousarray(
        W1f.T.reshape(6, 128, 576).transpose(1, 0, 2).reshape(128, 6 * 576))

    w2h = np.zeros((9, 5, 128, 64), np.float32)
    for dh in range(3):
        for dw in range(3):
            s = dh * 3 + dw
            T = iv["cnn1_w"][:, :, dh, dw].T.astype(np.float32)  # (576, 64)
            Tp = np.zeros((640, 64), np.float32)
            Tp[:576] = T
            w2h[s] = Tp.reshape(5, 128, 64)
    w["w2"] = np.ascontiguousarray(
        w2h.transpose(2, 0, 1, 3).reshape(128, 45 * 64))

    for name, key in (("wih", "w_ih"), ("whh", "w_hh")):
        T = iv[key].T.astype(np.float32)  # (256, 768)
        w[name] = np.ascontiguousarray(
            T.reshape(2, 128, 768).transpose(1, 0, 2).reshape(128, 1536))
    T = iv["fi_w"].T.astype(np.float32)  # (256, 256)
    w["fiw"] = np.ascontiguousarray(
        T.reshape(2, 128, 256).transpose(1, 0, 2).reshape(128, 512))
    T = iv["fn_w"].T.astype(np.float32)  # (256, 2)
    w["fnw"] = np.ascontiguousarray(
        T.reshape(2, 128, 2).transpose(1, 0, 2).reshape(128, 4))
    return w


def _prep_smalls(iv, x, core):
    sm = np.zeros((128, SM_COLS), np.float32)

    def put(name, arr):
        rows, off, width = SM_LAYOUT[name]
        a = np.asarray(arr, np.float32).reshape(rows, width)
        sm[0:rows, off:off + width] = a

    # conv2 position-dependent bias fold (conv1 bias + cnn1_b)
    M = np.einsum("oiab,i->oab", iv["cnn1_w"], iv["cnn_b"]).astype(np.float32)
    B2 = np.zeros((64, 7, 7), np.float32)
    for ph in range(7):
        for pw in range(7):
            acc = iv["cnn1_b"].astype(np.float32).copy()
            for dh in range(3):
                for dw in range(3):
                    if 0 <= ph + dh - 1 <= 6 and 0 <= pw + dw - 1 <= 6:
                        acc = acc + M[:, dh, dw]
            B2[:, ph, pw] = acc
    put("b2t", np.tile(B2.reshape(64, 49), (1, FPG)))

    inv = iv["bn_g"] / np.sqrt(iv["bn_v"] + BN_EPS)
    put("pscale", (inv / 49.0)[:, None])
    put("pshift", (iv["bn_b"] - iv["bn_m"] * inv)[:, None])

    xb = x[core * BPC:(core + 1) * BPC]  # (4, 16, 12)
    put("xt", xb.transpose(2, 1, 0).reshape(12, 64))

    put("a0t", iv["a0_w"].T)
    put("a0b", iv["a0_b"][:, None])
    put("ait", iv["ai_w"].T)
    put("aib", iv["ai_b"][:, None])
    put("ants2", iv["an_w"][:, 0:16].T)
    put("antf", iv["an_w"][:, 16:80].T)
    put("anb", iv["an_b"].reshape(2, 128).T)

    bs = (iv["b_ih"] + iv["b_hh"]).astype(np.float32)
    bs[512:] = iv["b_ih"][512:]
    put("bsgi", bs.reshape(6, 128).T)
    put("bsdec", np.repeat(bs.reshape(6, 128).T, 4, axis=1))
    put("bhhn", np.repeat(iv["b_hh"][512:].reshape(2, 128).T, 4, axis=1))
    put("fib", np.repeat(iv["fi_b"].reshape(2, 128).T, 4, axis=1))
    put("fnb", iv["fn_b"][:, None])
    return sm


def make_in_maps(inputs):
    iv = {k: np.asarray(v, np.float32) for k, v in inputs.items()}
    frames = iv["frames"]
    x = iv["x"]
    fr_all = _prep_frames(frames)
    w = _prep_weights(iv)
    in_maps = []
    for c in range(NCORES):
        m = {"fr": np.ascontiguousarray(fr_all[c]),
             "smalls": _prep_smalls(iv, x, c)}
        m.update(w)
        in_maps.append(m)
    return in_maps


_NC_CACHE = None


def get_nc():
    global _NC_CACHE
    if _NC_CACHE is None:
        _NC_CACHE = build_nc()
    return _NC_CACHE


def _install_ntff_hook():
    """The agent image's antenv lacks axon_hooks; synthesize it so
    run_bass_kernel_spmd(trace=True) can capture NTFF profiles."""
    try:
        from antenv.axon_hooks import get_axon_ntff_profile_hook  # noqa: F401
        return True
    except ImportError:
        pass
    try:
        import types
        import antenv
        if "/root/.axon_site" not in sys.path:
            sys.path.insert(0, "/root/.axon_site")
        from trn_agent_boot.trn_boot import _ntff_profile_via_ctypes
        hook = _ntff_profile_via_ctypes("/opt/axon/libaxon_pjrt.so")
        mod = types.ModuleType("antenv.axon_hooks")
        mod.get_axon_ntff_profile_hook = lambda: hook
        mod.set_axon_ntff_profile_hook = lambda h: None
        sys.modules["antenv.axon_hooks"] = mod
        antenv.axon_hooks = mod
        return hook is not None
    except Exception as e:  # pragma: no cover - profiling is best-effort
        print(f"ntff hook install failed: {e}")
        return False


def kernel(**inputs):
    global LAST_EXEC_NS, LAST_RESULTS
    nc = get_nc()
    in_maps = make_in_maps(inputs)
    trace = bool(int(os.environ.get("KERNEL_TRACE", "0")))
    if trace:
        trace = _install_ntff_hook()
    res = run_bass_kernel_spmd(nc, in_maps, core_ids=list(range(NCORES)),
                               trace=trace)
    LAST_RESULTS = res
    LAST_EXEC_NS = res.exec_time_ns
    outs = []
    for c in range(NCORES):
        o = res.results[c]["out"]  # (2, 40)
        outs.append(o.reshape(2, HOR, BPC).transpose(1, 2, 0)[:, :, None, :])
    return np.concatenate(outs, axis=1).astype(np.float32)


if __name__ == "__main__":
    nc = get_nc()
    print("built ok; instructions:",
          sum(len(bb.instructions) for bb in nc.main_func.blocks))


# revision 28
# speedup vs baseline: 2.5627x; 1.0194x over previous
"""CRNN Trainium2 kernel: patchify-conv -> 3x3 conv -> pool -> GRU encoder ->
autoregressive GRU decoder. Pure data-parallel over batch (32 -> 8 cores x 4).

Host side: numpy layout transforms (patch-transpose of frames, weight
re-layouts, BN/bias folding). Device side: one Bass/Tile SPMD program run on 8
NeuronCores via run_bass_kernel_spmd.
"""

import os
import sys

for _p in ("/opt/trn_rl_repo", "/root/.axon_site/_ro/trn_rl_repo"):
    if os.path.isdir(_p) and _p not in sys.path:
        sys.path.insert(0, _p)

import numpy as np

import concourse.bass as bass  # noqa: E402
import concourse.mybir as mybir  # noqa: E402
import concourse.tile as tile  # noqa: E402
from concourse import bacc  # noqa: E402
from concourse.bass_utils import run_bass_kernel_spmd  # noqa: E402

F32 = mybir.dt.float32
AF = mybir.ActivationFunctionType

# Model dims (hardcoded from the problem spec)
B, L, DS, DA, DC, DR, DO, HOR = 32, 16, 12, 16, 64, 256, 2, 10
NCORES, BPC = 8, 4          # batch per core
NG, FPG = 8, 8              # 8 groups of 8 frames per core (frame idx = l*4+b)
NPOS = 49                   # 7x7 patch grid
BN_EPS = 1e-5

# 'f32r' = single-pass full-rate fp32 matmul (reduced internal precision),
# 'f32'  = exact two-pass fp32. Applied via AP bitcast at matmul sites only.
MM_DT_CONV = os.environ.get("BASS_MM_DT_CONV", "f32r")
MM_DT_RNN = os.environ.get("BASS_MM_DT_RNN", "f32r")
MM_DT_C2 = os.environ.get("BASS_C2_DT", "bf16")   # conv2 matmul dtype
C2_TILEPOS = os.environ.get("BASS_C2_TP", "1") == "1"


def _dt_of(tag):
    return {"f32": mybir.dt.float32, "f32r": mybir.dt.float32r,
            "bf16": mybir.dt.bfloat16, "f16": mybir.dt.float16}[tag]

LAST_EXEC_NS = None
LAST_RESULTS = None


def _smalls_layout():
    """Layout of the packed per-core 'smalls' tensor [128, ncols].
    Entries are (row0, rows, col0, width); *B entries are copies placed on
    partitions 64:128 for the conv2 column-tiled frame-half pipeline."""
    out = {}
    cols = 0

    def add(name, rows, width, row0=0, share=None):
        nonlocal cols
        c0 = out[share][2] if share else cols
        out[name] = (row0, rows, c0, width)
        if not share:
            cols += width

    add("b2t", 64, 392)      # conv2 bias table (64, 49) tiled x8 frames
    add("b2tB", 64, 392, row0=64, share="b2t")
    add("pscale", 64, 1)     # pool+BN scale
    add("pscaleB", 64, 1, row0=64, share="pscale")
    add("pshift", 64, 1)     # pool+BN shift
    add("pshiftB", 64, 1, row0=64, share="pshift")
    add("xt", 12, 64)        # per-core x transposed, col = l*4+b
    add("a0t", 12, 16)
    add("a0b", 16, 1)
    add("ait", 16, 16)
    add("aib", 16, 1)
    add("ants2", 16, 256)    # an_w[:, :16].T
    add("antf", 64, 256)     # an_w[:, 16:].T
    add("antfB", 64, 256, row0=64, share="antf")
    add("anb", 128, 2)       # an_b chunks as cols
    add("bsgi", 128, 6)      # b_ih + b_hh (rz) / b_ih (n), chunk cols
    add("bsdec", 128, 24)    # same, tiled x4 batch cols
    add("bhhn", 128, 8)      # b_hh n-part, tiled x4
    add("fib", 128, 8)       # fi_b chunk cols tiled x4
    add("fnb", 2, 1)
    return out, cols


SM_LAYOUT, SM_COLS = _smalls_layout()

# conv2 shifts: center first so its start=True initializes the full PSUM rect.
SHIFTS = [(1, 1)] + [(dh, dw) for dh in range(3) for dw in range(3)
                     if (dh, dw) != (1, 1)]


def build_nc():
    nc = bacc.Bacc("TRN2", target_bir_lowering=False, debug=False,
                   num_devices=NCORES)
    mm_conv_g = mybir.dt.float32r if MM_DT_CONV == "f32r" else F32
    mm_rnn_g = mybir.dt.float32r if MM_DT_RNN == "f32r" else F32

    h_fr = nc.dram_tensor("fr", [NG, 128, 6 * 392], mm_conv_g, kind="ExternalInput")
    h_sm = nc.dram_tensor("smalls", [128, SM_COLS], mm_rnn_g, kind="ExternalInput")
    h_w1 = nc.dram_tensor("w1", [128, 6 * 576], mm_conv_g, kind="ExternalInput")
    h_w2 = nc.dram_tensor("w2", [128, 45 * 64], _dt_of(MM_DT_C2),
                          kind="ExternalInput")
    h_wih = nc.dram_tensor("wih", [128, 2 * 768], mm_rnn_g, kind="ExternalInput")
    h_whh = nc.dram_tensor("whh", [128, 2 * 768], mm_rnn_g, kind="ExternalInput")
    h_fi = nc.dram_tensor("fiw", [128, 2 * 256], mm_rnn_g, kind="ExternalInput")
    h_fn = nc.dram_tensor("fnw", [128, 4], mm_rnn_g, kind="ExternalInput")
    h_out = nc.dram_tensor("out", [2, 4 * HOR], F32, kind="ExternalOutput")
    debug = bool(int(os.environ.get("KERNEL_DEBUG", "0")))
    if debug:
        h_dsenc = nc.dram_tensor("d_senc", [128, 2, 64], F32,
                                 kind="ExternalOutput")
        h_dgi = nc.dram_tensor("d_gi", [128, 6, 64], F32,
                               kind="ExternalOutput")
        h_dh = nc.dram_tensor("d_h", [128, 8], F32, kind="ExternalOutput")
        h_df1 = nc.dram_tensor("d_f1", [128, 5, FPG, 9, 9], F32,
                               kind="ExternalOutput")
        h_dpg0 = nc.dram_tensor("d_pg0", [128, 48], F32,
                                kind="ExternalOutput")
        h_dhn0 = nc.dram_tensor("d_hn0", [128, 8], F32,
                                kind="ExternalOutput")
        h_dxr0 = nc.dram_tensor("d_xr0", [128, 8], F32,
                                kind="ExternalOutput")

    mm_conv = mybir.dt.float32r if MM_DT_CONV == "f32r" else F32
    mm_rnn = mybir.dt.float32r if MM_DT_RNN == "f32r" else F32
    MC = mm_conv   # dtype for conv matmul operand tensors
    MR = mm_rnn    # dtype for rnn matmul operand tensors

    mm_c2 = _dt_of(MM_DT_C2)

    def mmc(out, lhsT, rhs, **kw):
        nc.tensor.matmul(out, lhsT.bitcast(mm_conv), rhs.bitcast(mm_conv),
                         skip_group_check=True, **kw)

    def mmc2(out, lhsT, rhs, **kw):
        nc.tensor.matmul(out, lhsT.bitcast(mm_c2), rhs.bitcast(mm_c2),
                         skip_group_check=True, **kw)

    def mmr(out, lhsT, rhs, **kw):
        nc.tensor.matmul(out, lhsT.bitcast(mm_rnn), rhs.bitcast(mm_rnn),
                         skip_group_check=True, **kw)

    with tile.TileContext(nc) as tc:
        from contextlib import ExitStack
        with ExitStack() as ctx:
            cpool = ctx.enter_context(tc.tile_pool(name="const", bufs=1))
            xin_pool = ctx.enter_context(tc.tile_pool(name="xin", bufs=3))
            f1_pool = ctx.enter_context(tc.tile_pool(name="f1", bufs=2))
            work = ctx.enter_context(tc.tile_pool(name="work", bufs=4))
            state = ctx.enter_context(tc.tile_pool(name="state", bufs=1))
            hpool = ctx.enter_context(tc.tile_pool(name="h", bufs=3))
            ps1 = ctx.enter_context(
                tc.tile_pool(name="ps1", bufs=3, space="PSUM"))
            ps2 = ctx.enter_context(
                tc.tile_pool(name="ps2", bufs=1, space="PSUM"))
            psr = ctx.enter_context(
                tc.tile_pool(name="psr", bufs=3, space="PSUM"))

            # ---- constants to SBUF ----
            w1 = cpool.tile([128, 6 * 576], MC, tag="w1")
            nc.sync.dma_start(w1[:], h_w1[:])
            w2 = cpool.tile([128, 45 * 64], mm_c2, tag="w2")
            nc.sync.dma_start(w2[:], h_w2[:])
            wih = cpool.tile([128, 2 * 768], MR, tag="wih")
            nc.sync.dma_start(wih[:], h_wih[:])
            whh = cpool.tile([128, 2 * 768], MR, tag="whh")
            nc.sync.dma_start(whh[:], h_whh[:])
            fiw = cpool.tile([128, 2 * 256], MR, tag="fiw")
            nc.sync.dma_start(fiw[:], h_fi[:])
            fnw = cpool.tile([128, 4], MR, tag="fnw")
            nc.sync.dma_start(fnw[:], h_fn[:])
            sm = cpool.tile([128, SM_COLS], MR, tag="sm")
            nc.sync.dma_start(sm[:], h_sm[:])

            def sv(name):
                r0, rows, off, width = SM_LAYOUT[name]
                return sm[r0:r0 + rows, off:off + width]

            def svc(name, c0, w):  # column sub-slice of a smalls entry
                r0, rows, off, width = SM_LAYOUT[name]
                assert c0 + w <= width
                return sm[r0:r0 + rows, off + c0:off + c0 + w]

            def svf(name):  # fp32 view for ACT bias/scale and DVE operands
                return sv(name).bitcast(F32)

            def svcf(name, c0, w):
                return svc(name, c0, w).bitcast(F32)

            # ---- persistent state tiles ----
            # conv2 gutter-flat layout: 8x8 cells per frame (row 0 + col 0 of
            # each 8-cell row are zero gutters), LEAD/TAIL pads absorb shift
            # window spill. All 9 conv2 taps become contiguous 2D windows.
            LEAD, FB, FW = 16, 64, 16 + 8 * 64 + 16  # 544 per ic-chunk
            zf1 = state.tile([128, 5 * FW], F32, tag="zf1")
            nc.gpsimd.memset(zf1[:], 0.0)
            s2 = state.tile([16, 64], MR, tag="s2")
            s_enc = state.tile([128, 2, 64], MR, tag="senc")
            GI = state.tile([128, 6, 64], F32, tag="gi")
            preds = state.tile([2, 4 * HOR], F32, tag="preds")

            # ---- state adapters: s1 = relu(a0 x); s2 = s1 + relu(ai s1) ----
            pa = psr.tile([16, 64], F32, tag="ps")
            mmr(pa[:], sv("a0t"), sv("xt"), start=True, stop=True)
            s1 = work.tile([16, 64], MR, tag="s1")
            nc.scalar.activation(s1[:], pa[:], AF.Relu, bias=svf("a0b"))
            pb = psr.tile([16, 64], F32, tag="ps")
            mmr(pb[:], sv("ait"), s1[:], start=True, stop=True)
            s1b = work.tile([16, 64], MR, tag="s1")
            nc.scalar.activation(s1b[:], pb[:], AF.Relu, bias=svf("aib"))
            nc.vector.tensor_add(s2[:], s1[:], s1b[:])

            # encoder hidden state
            h_cur = hpool.tile([128, 8], MR, tag="h")
            nc.vector.tensor_copy(h_cur[:], zf1[:, 0:8])

            def enc_step(t, h_prev):
                pg = psr.tile([128, 24], F32, tag="ps")
                for mc in range(6):
                    for kc in range(2):
                        mmr(pg[:, mc * 4:(mc + 1) * 4],
                            whh[:, kc * 768 + mc * 128:kc * 768 + (mc + 1) * 128],
                            h_prev[:, kc * 4:(kc + 1) * 4],
                            start=(kc == 0), stop=(kc == 1))
                gi_rz = GI[:, 0:4, t * 4:(t + 1) * 4]
                gi_n = GI[:, 4:6, t * 4:(t + 1) * 4]
                pre = work.tile([128, 16], F32, tag="g16")
                nc.vector.tensor_add(
                    pre[:].rearrange("p (c b) -> p c b", b=4), gi_rz,
                    pg[:, 0:16].rearrange("p (c b) -> p c b", b=4))
                rz = work.tile([128, 16], F32, tag="g16b")
                nc.scalar.activation(rz[:], pre[:], AF.Sigmoid)
                a1 = work.tile([128, 8], F32, tag="g8")
                nc.vector.tensor_add(a1[:], pg[:, 16:24], svf("bhhn"))
                a2 = work.tile([128, 8], F32, tag="g8b")
                nc.vector.tensor_mul(a2[:], rz[:, 0:8], a1[:])
                a3 = work.tile([128, 8], F32, tag="g8c")
                nc.vector.tensor_add(
                    a3[:].rearrange("p (c b) -> p c b", b=4),
                    a2[:].rearrange("p (c b) -> p c b", b=4), gi_n)
                nt = work.tile([128, 8], F32, tag="g8d")
                nc.scalar.activation(nt[:], a3[:], AF.Tanh)
                hmn = work.tile([128, 8], F32, tag="g8e")
                nc.vector.tensor_sub(hmn[:], h_prev[:], nt[:])
                zt = work.tile([128, 8], F32, tag="g8f")
                nc.vector.tensor_mul(zt[:], rz[:, 8:16], hmn[:])
                h_new = hpool.tile([128, 8], MR, tag="h")
                nc.vector.tensor_add(h_new[:], nt[:], zt[:])
                return h_new

            # ---- conv + features + GI, per group of 8 frames ----
            for g in range(NG):
                xin = xin_pool.tile([128, 6 * 392], MC, tag="xin")
                nc.sync.dma_start(xin[:], h_fr[g])

                # f1 in gutter-flat layout: data (f, ph, pw) at col
                # LEAD + f*64 + (ph+1)*8 + (pw+1); gutters stay zero.
                f1 = f1_pool.tile([128, 5, FW], mm_c2, tag="f1")
                nc.vector.tensor_copy(
                    f1[:].rearrange("p a b -> p (a b)"), zf1[:])

                for m in range(5):
                    msz = 128 if m < 4 else 64
                    p1 = ps1.tile([msz, 392], F32, tag="c1")
                    for k in range(6):
                        mmc(p1[:],
                            w1[:, k * 576 + m * 128:k * 576 + m * 128 + msz],
                            xin[:, k * 392:(k + 1) * 392],
                            start=(k == 0), stop=(k == 5))
                    dst = f1[0:msz, m, LEAD:LEAD + 8 * FB].rearrange(
                        "p (f a b) -> p f a b", a=8, b=8)[:, :, 1:8, 1:8]
                    nc.vector.tensor_copy(
                        dst, p1[:].rearrange("p (f a b) -> p f a b", a=7, b=7))

                # conv2: 9 taps x 5 ic-chunks, frame-half 0 on PE column half
                # 0 (out partitions 0:64) and frame-half 1 on column half 1
                # (out partitions 64:128) — the two run concurrently.
                p2 = ps2.tile([128, 256], F32, tag="c2", name=f"p2_{g}")
                for si, (dh, dw) in enumerate(SHIFTS):
                    s = dh * 3 + dw
                    delta = (dh - 1) * 8 + (dw - 1)
                    for k in range(5):
                        for h in (0, 1):
                            a = LEAD + h * 256 + delta
                            mmc2(p2[64 * h:64 * h + 64, :],
                                 w2[:, (s * 5 + k) * 64:(s * 5 + k + 1) * 64],
                                 f1[:, k, a:a + 256],
                                 start=(si == 0 and k == 0),
                                 stop=(si == len(SHIFTS) - 1 and k == 4),
                                 tile_position=(0, 64 * h)
                                 if C2_TILEPOS else None)

                # epilogue: relu(conv2 + B2) -> mean -> BN affine, per half
                # (half h lives on partitions 64h:64h+64 throughout)
                red = work.tile([128, 4], F32, tag="red")
                feats = work.tile([128, 4], MR, tag="feats")
                for h, b2n, psn, pshn in ((0, "b2t", "pscale", "pshift"),
                                          (1, "b2tB", "pscaleB", "pshiftB")):
                    r0 = 64 * h
                    pv = p2[r0:r0 + 64, :].rearrange(
                        "p (f a b) -> p f a b", a=8, b=8)[:, :, 1:8, 1:8]
                    b2v = svcf(b2n, h * 196, 196).rearrange(
                        "p (f a b) -> p f a b", a=7, b=7)
                    t0 = work.tile([128, 4, 7, 7], F32, tag="ep")
                    nc.vector.tensor_add(t0[r0:r0 + 64], pv, b2v)
                    t1 = work.tile([128, 4, 7, 7], F32, tag="ep2")
                    nc.vector.tensor_scalar_max(t1[r0:r0 + 64],
                                                t0[r0:r0 + 64], 0.0)
                    nc.vector.tensor_reduce(
                        red[r0:r0 + 64, :], t1[r0:r0 + 64],
                        axis=mybir.AxisListType.XY, op=mybir.AluOpType.add)
                    nc.scalar.activation(feats[r0:r0 + 64, :],
                                         red[r0:r0 + 64, :], AF.Identity,
                                         bias=svf(pshn), scale=svf(psn))

                # an: relu(an_w [s2; feats] + an_b); K split 64(feats)+16(s2),
                # N split per frame-half (feats halves live on different
                # partition ranges)
                for mc in range(2):
                    pan = psr.tile([128, FPG], F32, tag="ps")
                    for h, antn in ((0, "antf"), (1, "antfB")):
                        cs = slice(h * 4, h * 4 + 4)
                        gc = slice(g * FPG + h * 4, g * FPG + h * 4 + 4)
                        mmr(pan[:, cs], svc(antn, mc * 128, 128),
                            feats[64 * h:64 * h + 64, :],
                            start=True, stop=False)
                        mmr(pan[:, cs], svc("ants2", mc * 128, 128),
                            s2[:, gc], start=False, stop=True)
                    gcol = slice(g * FPG, (g + 1) * FPG)
                    nc.scalar.activation(s_enc[:, mc, gcol], pan[:], AF.Relu,
                                         bias=svcf("anb", mc, 1))

                # GI = w_ih @ s_enc + (b_ih + b_hh fold) for these 8 cols
                for mc in range(6):
                    pgi = psr.tile([128, FPG], F32, tag="ps")
                    for kc in range(2):
                        mmr(pgi[:],
                            wih[:, kc * 768 + mc * 128:kc * 768 + (mc + 1) * 128],
                            s_enc[:, kc, gcol],
                            start=(kc == 0), stop=(kc == 1))
                    nc.scalar.activation(GI[:, mc, gcol], pgi[:], AF.Identity,
                                         bias=svcf("bsgi", mc, 1))

                if debug and g == 0:
                    nc.sync.dma_start(h_df1[:], f1[:])

                # encoder steps that become ready after this group
                h_cur = enc_step(2 * g, h_cur)
                h_cur = enc_step(2 * g + 1, h_cur)

            if debug:
                nc.sync.dma_start(h_dsenc[:], s_enc[:])
                nc.sync.dma_start(h_dgi[:], GI[:])
                nc.sync.dma_start(h_dh[:], h_cur[:])

            # ---- decoder ----
            xi, hh = h_cur, h_cur
            for t in range(HOR):
                pg = psr.tile([128, 48], F32, tag="ps")
                # each PSUM accumulation group must complete before the next
                # one starts (interleaved groups break accumulation)
                for mc in range(6):
                    for kc in range(2):
                        mmr(pg[:, mc * 4:(mc + 1) * 4],
                            wih[:, kc * 768 + mc * 128:
                                kc * 768 + (mc + 1) * 128],
                            xi[:, kc * 4:(kc + 1) * 4],
                            start=(kc == 0), stop=(kc == 1))
                    for kc in range(2):
                        mmr(pg[:, 24 + mc * 4:24 + (mc + 1) * 4],
                            whh[:, kc * 768 + mc * 128:
                                kc * 768 + (mc + 1) * 128],
                            hh[:, kc * 4:(kc + 1) * 4],
                            start=(kc == 0), stop=(kc == 1))
                if debug and t == 0:
                    pgc = work.tile([128, 48], F32, tag="dbgpg")
                    nc.vector.tensor_copy(pgc[:], pg[:])
                    nc.sync.dma_start(h_dpg0[:], pgc[:])
                gisb = work.tile([128, 24], F32, tag="g24")
                nc.vector.tensor_add(gisb[:], pg[:, 0:24], svf("bsdec"))
                pre = work.tile([128, 16], F32, tag="g16")
                nc.vector.tensor_add(pre[:], gisb[:, 0:16], pg[:, 24:40])
                rz = work.tile([128, 16], F32, tag="g16b")
                nc.scalar.activation(rz[:], pre[:], AF.Sigmoid)
                a1 = work.tile([128, 8], F32, tag="g8")
                nc.vector.tensor_add(a1[:], pg[:, 40:48], svf("bhhn"))
                a2 = work.tile([128, 8], F32, tag="g8b")
                nc.vector.tensor_mul(a2[:], rz[:, 0:8], a1[:])
                a3 = work.tile([128, 8], F32, tag="g8c")
                nc.vector.tensor_add(a3[:], a2[:], gisb[:, 16:24])
                nt = work.tile([128, 8], F32, tag="g8d")
                nc.scalar.activation(nt[:], a3[:], AF.Tanh)
                hmn = work.tile([128, 8], F32, tag="g8e")
                nc.vector.tensor_sub(hmn[:], hh[:], nt[:])
                zt = work.tile([128, 8], F32, tag="g8f")
                nc.vector.tensor_mul(zt[:], rz[:, 8:16], hmn[:])
                hn = hpool.tile([128, 8], MR, tag="h")
                nc.vector.tensor_add(hn[:], nt[:], zt[:])

                pfi = psr.tile([128, 8], F32, tag="ps")
                for mc2 in range(2):
                    for kc2 in range(2):
                        mmr(pfi[:, mc2 * 4:(mc2 + 1) * 4],
                            fiw[:, kc2 * 256 + mc2 * 128:
                                kc2 * 256 + (mc2 + 1) * 128],
                            hn[:, kc2 * 4:(kc2 + 1) * 4],
                            start=(kc2 == 0), stop=(kc2 == 1))
                u1 = work.tile([128, 8], F32, tag="g8")
                nc.vector.tensor_add(u1[:], pfi[:], svf("fib"))
                u2 = work.tile([128, 8], F32, tag="g8b")
                nc.vector.tensor_scalar_max(u2[:], u1[:], 0.0)
                xr = hpool.tile([128, 8], MR, tag="xr")
                nc.vector.tensor_add(xr[:], hn[:], u2[:])

                pfn = psr.tile([2, 4], F32, tag="ps")
                for kc in range(2):
                    mmr(pfn[:], fnw[:, kc * 2:(kc + 1) * 2],
                        xr[:, kc * 4:(kc + 1) * 4],
                        start=(kc == 0), stop=(kc == 1))
                nc.scalar.activation(preds[:, t * 4:(t + 1) * 4], pfn[:],
                                     AF.Tanh, bias=svf("fnb"))
                if debug and t == 0:
                    nc.sync.dma_start(h_dhn0[:], hn[:])
                    nc.sync.dma_start(h_dxr0[:], xr[:])
                xi, hh = xr, hn

            nc.sync.dma_start(h_out[:], preds[:])

    nc.finalize()
    return nc


# ---------------- host-side data prep ----------------

def _prep_frames(frames):
    """frames (32,16,3,112,112) -> per-core [NG, 128, 2352] patch-T layout."""
    out = np.empty((NCORES, NG, 128, 6 * 392), np.float32)
    fr = np.ascontiguousarray(frames, np.float32)
    for c in range(NCORES):
        fb = fr[c * BPC:(c + 1) * BPC]  # (4, 16, 3, 112, 112)
        a = fb.reshape(BPC, L, 3, 7, 16, 7, 16)
        # -> [l, b, ch, kh, kw, ph, pw]
        a = a.transpose(1, 0, 2, 4, 6, 3, 5)
        a = a.reshape(L, BPC, 768, 49)
        a = a.reshape(NG, 2, BPC, 6, 128, 49)
        # -> [g, k, p, li, b, s]
        a = a.transpose(0, 3, 4, 1, 2, 5)
        a = a.reshape(NG, 6, 128, 392)
        a = a.transpose(0, 2, 1, 3)  # [g, p, k, 392]
        out[c] = a.reshape(NG, 128, 6 * 392)
    return out


def _prep_weights(iv):
    w = {}
    W1f = iv["cnn_w"].reshape(576, 768).astype(np.float32)
    w["w1"] = np.ascontiguousarray(
        W1f.T.reshape(6, 128, 576).transpose(1, 0, 2).reshape(128, 6 * 576))

    w2h = np.zeros((9, 5, 128, 64), np.float32)
    for dh in range(3):
        for dw in range(3):
            s = dh * 3 + dw
            T = iv["cnn1_w"][:, :, dh, dw].T.astype(np.float32)  # (576, 64)
            Tp = np.zeros((640, 64), np.float32)
            Tp[:576] = T
            w2h[s] = Tp.reshape(5, 128, 64)
    w["w2"] = np.ascontiguousarray(
        w2h.transpose(2, 0, 1, 3).reshape(128, 45 * 64)).astype(
            mybir.dt.np(_dt_of(MM_DT_C2)))

    for name, key in (("wih", "w_ih"), ("whh", "w_hh")):
        T = iv[key].T.astype(np.float32)  # (256, 768)
        w[name] = np.ascontiguousarray(
            T.reshape(2, 128, 768).transpose(1, 0, 2).reshape(128, 1536))
    T = iv["fi_w"].T.astype(np.float32)  # (256, 256)
    w["fiw"] = np.ascontiguousarray(
        T.reshape(2, 128, 256).transpose(1, 0, 2).reshape(128, 512))
    T = iv["fn_w"].T.astype(np.float32)  # (256, 2)
    w["fnw"] = np.ascontiguousarray(
        T.reshape(2, 128, 2).transpose(1, 0, 2).reshape(128, 4))
    return w


def _prep_smalls(iv, x, core):
    sm = np.zeros((128, SM_COLS), np.float32)

    def put(name, arr):
        r0, rows, off, width = SM_LAYOUT[name]
        a = np.asarray(arr, np.float32).reshape(rows, width)
        sm[r0:r0 + rows, off:off + width] = a

    # conv2 position-dependent bias fold (conv1 bias + cnn1_b)
    M = np.einsum("oiab,i->oab", iv["cnn1_w"], iv["cnn_b"]).astype(np.float32)
    B2 = np.zeros((64, 7, 7), np.float32)
    for ph in range(7):
        for pw in range(7):
            acc = iv["cnn1_b"].astype(np.float32).copy()
            for dh in range(3):
                for dw in range(3):
                    if 0 <= ph + dh - 1 <= 6 and 0 <= pw + dw - 1 <= 6:
                        acc = acc + M[:, dh, dw]
            B2[:, ph, pw] = acc
    put("b2t", np.tile(B2.reshape(64, 49), (1, FPG)))
    put("b2tB", np.tile(B2.reshape(64, 49), (1, FPG)))

    inv = iv["bn_g"] / np.sqrt(iv["bn_v"] + BN_EPS)
    put("pscale", (inv / 49.0)[:, None])
    put("pscaleB", (inv / 49.0)[:, None])
    put("pshift", (iv["bn_b"] - iv["bn_m"] * inv)[:, None])
    put("pshiftB", (iv["bn_b"] - iv["bn_m"] * inv)[:, None])

    xb = x[core * BPC:(core + 1) * BPC]  # (4, 16, 12)
    put("xt", xb.transpose(2, 1, 0).reshape(12, 64))

    put("a0t", iv["a0_w"].T)
    put("a0b", iv["a0_b"][:, None])
    put("ait", iv["ai_w"].T)
    put("aib", iv["ai_b"][:, None])
    put("ants2", iv["an_w"][:, 0:16].T)
    put("antf", iv["an_w"][:, 16:80].T)
    put("antfB", iv["an_w"][:, 16:80].T)
    put("anb", iv["an_b"].reshape(2, 128).T)

    bs = (iv["b_ih"] + iv["b_hh"]).astype(np.float32)
    bs[512:] = iv["b_ih"][512:]
    put("bsgi", bs.reshape(6, 128).T)
    put("bsdec", np.repeat(bs.reshape(6, 128).T, 4, axis=1))
    put("bhhn", np.repeat(iv["b_hh"][512:].reshape(2, 128).T, 4, axis=1))
    put("fib", np.repeat(iv["fi_b"].reshape(2, 128).T, 4, axis=1))
    put("fnb", iv["fn_b"][:, None])
    return sm


def make_in_maps(inputs):
    iv = {k: np.asarray(v, np.float32) for k, v in inputs.items()}
    frames = iv["frames"]
    x = iv["x"]
    fr_all = _prep_frames(frames)
    w = _prep_weights(iv)
    in_maps = []
    for c in range(NCORES):
        m = {"fr": np.ascontiguousarray(fr_all[c]),
             "smalls": _prep_smalls(iv, x, c)}
        m.update(w)
        in_maps.append(m)
    return in_maps


_NC_CACHE = None


def get_nc():
    global _NC_CACHE
    if _NC_CACHE is None:
        _NC_CACHE = build_nc()
    return _NC_CACHE


def _install_ntff_hook():
    """The agent image's antenv lacks axon_hooks; synthesize it so
    run_bass_kernel_spmd(trace=True) can capture NTFF profiles."""
    try:
        from antenv.axon_hooks import get_axon_ntff_profile_hook  # noqa: F401
        return True
    except ImportError:
        pass
    try:
        import types
        import antenv
        if "/root/.axon_site" not in sys.path:
            sys.path.insert(0, "/root/.axon_site")
        from trn_agent_boot.trn_boot import _ntff_profile_via_ctypes
        hook = _ntff_profile_via_ctypes("/opt/axon/libaxon_pjrt.so")
        mod = types.ModuleType("antenv.axon_hooks")
        mod.get_axon_ntff_profile_hook = lambda: hook
        mod.set_axon_ntff_profile_hook = lambda h: None
        sys.modules["antenv.axon_hooks"] = mod
        antenv.axon_hooks = mod
        return hook is not None
    except Exception as e:  # pragma: no cover - profiling is best-effort
        print(f"ntff hook install failed: {e}")
        return False


def kernel(**inputs):
    global LAST_EXEC_NS, LAST_RESULTS
    nc = get_nc()
    in_maps = make_in_maps(inputs)
    trace = bool(int(os.environ.get("KERNEL_TRACE", "0")))
    if trace:
        trace = _install_ntff_hook()
    res = run_bass_kernel_spmd(nc, in_maps, core_ids=list(range(NCORES)),
                               trace=trace)
    LAST_RESULTS = res
    LAST_EXEC_NS = res.exec_time_ns
    outs = []
    for c in range(NCORES):
        o = res.results[c]["out"]  # (2, 40)
        outs.append(o.reshape(2, HOR, BPC).transpose(1, 2, 0)[:, :, None, :])
    return np.concatenate(outs, axis=1).astype(np.float32)


if __name__ == "__main__":
    nc = get_nc()
    print("built ok; instructions:",
          sum(len(bb.instructions) for bb in nc.main_func.blocks))


# revision 31
# speedup vs baseline: 3.8307x; 1.4948x over previous
"""CRNN Trainium2 kernel: patchify-conv -> 3x3 conv -> pool -> GRU encoder ->
autoregressive GRU decoder. Pure data-parallel over batch (32 -> 8 cores x 4).

Host side: numpy layout transforms (patch-transpose of frames, weight
re-layouts, BN/bias folding). Device side: one Bass/Tile SPMD program run on 8
NeuronCores via run_bass_kernel_spmd.
"""

import os
import sys

for _p in ("/opt/trn_rl_repo", "/root/.axon_site/_ro/trn_rl_repo"):
    if os.path.isdir(_p) and _p not in sys.path:
        sys.path.insert(0, _p)

import numpy as np

import concourse.bass as bass  # noqa: E402
import concourse.mybir as mybir  # noqa: E402
import concourse.tile as tile  # noqa: E402
from concourse import bacc  # noqa: E402
from concourse.bass_utils import run_bass_kernel_spmd  # noqa: E402

F32 = mybir.dt.float32
AF = mybir.ActivationFunctionType

# Model dims (hardcoded from the problem spec)
B, L, DS, DA, DC, DR, DO, HOR = 32, 16, 12, 16, 64, 256, 2, 10
NCORES, BPC = 8, 4          # batch per core
NG, FPG = 8, 8              # 8 groups of 8 frames per core (frame idx = l*4+b)
NPOS = 49                   # 7x7 patch grid
BN_EPS = 1e-5

# 'f32r' = single-pass full-rate fp32 matmul (reduced internal precision),
# 'f32'  = exact two-pass fp32. Applied via AP bitcast at matmul sites only.
MM_DT_CONV = os.environ.get("BASS_MM_DT_CONV", "f32r")
MM_DT_RNN = os.environ.get("BASS_MM_DT_RNN", "f16")
MM_DT_C2 = os.environ.get("BASS_C2_DT", "bf16")   # conv2 matmul dtype
C2_TILEPOS = os.environ.get("BASS_C2_TP", "1") == "1"


def _dt_of(tag):
    return {"f32": mybir.dt.float32, "f32r": mybir.dt.float32r,
            "bf16": mybir.dt.bfloat16, "f16": mybir.dt.float16}[tag]

LAST_EXEC_NS = None
LAST_RESULTS = None


def _layout(entries):
    """entries: (name, rows, width[, row0, share]) -> dict + total cols.
    Entry = (row0, rows, col0, width); share reuses another entry's columns
    (the copy lives on a different partition range)."""
    out = {}
    cols = 0
    for e in entries:
        name, rows, width = e[0], e[1], e[2]
        row0 = e[3] if len(e) > 3 else 0
        share = e[4] if len(e) > 4 else None
        c0 = out[share][2] if share else cols
        out[name] = (row0, rows, c0, width)
        if not share:
            cols += width
    return out, cols


# matmul operands (RNN matmul dtype)
SMM_LAYOUT, SMM_COLS = _layout([
    ("xt", 12, 64),          # per-core x transposed, col = l*4+b
    ("a0t", 12, 16),
    ("ait", 16, 16),
    ("ants2", 16, 256),      # an_w[:, :16].T
    ("antf", 64, 256),       # an_w[:, 16:].T
    ("antfB", 64, 256, 64, "antf"),
])
# bias/affine tables (always fp32)
SMB_LAYOUT, SMB_COLS = _layout([
    ("b2t", 64, 392),        # conv2 bias table (64, 49) tiled x8 frames
    ("b2tB", 64, 392, 64, "b2t"),
    ("pscale", 64, 1),
    ("pscaleB", 64, 1, 64, "pscale"),
    ("pshift", 64, 1),
    ("pshiftB", 64, 1, 64, "pshift"),
    ("a0b", 16, 1),
    ("aib", 16, 1),
    ("anb", 128, 2),         # an_b chunks as cols
    ("bsgi", 128, 6),        # b_ih + b_hh (rz) / b_ih (n), chunk cols
    ("bsdec", 128, 24),      # same, tiled x4 batch cols
    ("bhhn", 128, 8),        # b_hh n-part, tiled x4
    ("fib", 128, 8),         # fi_b chunk cols tiled x4
    ("fnb", 2, 1),
])

# conv2 shifts: center first so its start=True initializes the full PSUM rect.
SHIFTS = [(1, 1)] + [(dh, dw) for dh in range(3) for dw in range(3)
                     if (dh, dw) != (1, 1)]


def build_nc():
    nc = bacc.Bacc("TRN2", target_bir_lowering=False, debug=False,
                   num_devices=NCORES)
    mm_conv_g = _dt_of(MM_DT_CONV)
    mm_rnn_g = _dt_of(MM_DT_RNN)

    h_fr = nc.dram_tensor("fr", [NG, 128, 6 * 392], mm_conv_g, kind="ExternalInput")
    h_smm = nc.dram_tensor("smm", [128, SMM_COLS], mm_rnn_g,
                           kind="ExternalInput")
    h_smb = nc.dram_tensor("smb", [128, SMB_COLS], F32, kind="ExternalInput")
    h_w1 = nc.dram_tensor("w1", [128, 6 * 576], mm_conv_g, kind="ExternalInput")
    h_w2 = nc.dram_tensor("w2", [128, 45 * 64], _dt_of(MM_DT_C2),
                          kind="ExternalInput")
    h_wih = nc.dram_tensor("wih", [128, 2 * 768], mm_rnn_g, kind="ExternalInput")
    h_whh = nc.dram_tensor("whh", [128, 2 * 768], mm_rnn_g, kind="ExternalInput")
    h_fi = nc.dram_tensor("fiw", [128, 2 * 256], mm_rnn_g, kind="ExternalInput")
    h_fn = nc.dram_tensor("fnw", [128, 4], mm_rnn_g, kind="ExternalInput")
    h_out = nc.dram_tensor("out", [2, 4 * HOR], F32, kind="ExternalOutput")
    debug = bool(int(os.environ.get("KERNEL_DEBUG", "0")))
    if debug:
        h_dsenc = nc.dram_tensor("d_senc", [128, 2, 64], F32,
                                 kind="ExternalOutput")
        h_dgi = nc.dram_tensor("d_gi", [128, 6, 64], F32,
                               kind="ExternalOutput")
        h_dh = nc.dram_tensor("d_h", [128, 8], F32, kind="ExternalOutput")
        h_df1 = nc.dram_tensor("d_f1", [128, 5, FPG, 9, 9], F32,
                               kind="ExternalOutput")
        h_dpg0 = nc.dram_tensor("d_pg0", [128, 48], F32,
                                kind="ExternalOutput")
        h_dhn0 = nc.dram_tensor("d_hn0", [128, 8], F32,
                                kind="ExternalOutput")
        h_dxr0 = nc.dram_tensor("d_xr0", [128, 8], F32,
                                kind="ExternalOutput")

    mm_conv = _dt_of(MM_DT_CONV)
    mm_rnn = _dt_of(MM_DT_RNN)
    MC = mm_conv   # dtype for conv matmul operand tensors
    MR = mm_rnn    # dtype for rnn matmul operand tensors

    mm_c2 = _dt_of(MM_DT_C2)

    def mmc(out, lhsT, rhs, **kw):
        nc.tensor.matmul(out, lhsT.bitcast(mm_conv), rhs.bitcast(mm_conv),
                         skip_group_check=True, **kw)

    def mmc2(out, lhsT, rhs, **kw):
        nc.tensor.matmul(out, lhsT.bitcast(mm_c2), rhs.bitcast(mm_c2),
                         skip_group_check=True, **kw)

    def mmr(out, lhsT, rhs, **kw):
        nc.tensor.matmul(out, lhsT.bitcast(mm_rnn), rhs.bitcast(mm_rnn),
                         skip_group_check=True, **kw)

    with tile.TileContext(nc) as tc:
        from contextlib import ExitStack
        with ExitStack() as ctx:
            cpool = ctx.enter_context(tc.tile_pool(name="const", bufs=1))
            xin_pool = ctx.enter_context(tc.tile_pool(name="xin", bufs=3))
            f1_pool = ctx.enter_context(tc.tile_pool(name="f1", bufs=2))
            work = ctx.enter_context(tc.tile_pool(name="work", bufs=4))
            state = ctx.enter_context(tc.tile_pool(name="state", bufs=1))
            hpool = ctx.enter_context(tc.tile_pool(name="h", bufs=3))
            ps1 = ctx.enter_context(
                tc.tile_pool(name="ps1", bufs=3, space="PSUM"))
            ps2 = ctx.enter_context(
                tc.tile_pool(name="ps2", bufs=1, space="PSUM"))
            psr = ctx.enter_context(
                tc.tile_pool(name="psr", bufs=3, space="PSUM"))

            # ---- constants to SBUF ----
            w1 = cpool.tile([128, 6 * 576], MC, tag="w1")
            nc.sync.dma_start(w1[:], h_w1[:])
            w2 = cpool.tile([128, 45 * 64], mm_c2, tag="w2")
            nc.sync.dma_start(w2[:], h_w2[:])
            wih = cpool.tile([128, 2 * 768], MR, tag="wih")
            nc.sync.dma_start(wih[:], h_wih[:])
            whh = cpool.tile([128, 2 * 768], MR, tag="whh")
            nc.sync.dma_start(whh[:], h_whh[:])
            fiw = cpool.tile([128, 2 * 256], MR, tag="fiw")
            nc.sync.dma_start(fiw[:], h_fi[:])
            fnw = cpool.tile([128, 4], MR, tag="fnw")
            nc.sync.dma_start(fnw[:], h_fn[:])
            smm = cpool.tile([128, SMM_COLS], MR, tag="smm")
            nc.sync.dma_start(smm[:], h_smm[:])
            smb = cpool.tile([128, SMB_COLS], F32, tag="smb")
            nc.sync.dma_start(smb[:], h_smb[:])

            def sv(name):  # matmul-operand view (RNN dtype)
                r0, rows, off, width = SMM_LAYOUT[name]
                return smm[r0:r0 + rows, off:off + width]

            def svc(name, c0, w):
                r0, rows, off, width = SMM_LAYOUT[name]
                assert c0 + w <= width
                return smm[r0:r0 + rows, off + c0:off + c0 + w]

            def svf(name):  # fp32 bias/affine view
                r0, rows, off, width = SMB_LAYOUT[name]
                return smb[r0:r0 + rows, off:off + width]

            def svcf(name, c0, w):
                r0, rows, off, width = SMB_LAYOUT[name]
                assert c0 + w <= width
                return smb[r0:r0 + rows, off + c0:off + c0 + w]

            # ---- persistent state tiles ----
            # conv2 gutter-flat layout: 8x8 cells per frame (row 0 + col 0 of
            # each 8-cell row are zero gutters), LEAD/TAIL pads absorb shift
            # window spill. All 9 conv2 taps become contiguous 2D windows.
            LEAD, FB, FW = 16, 64, 16 + 8 * 64 + 16  # 544 per ic-chunk
            zf1 = state.tile([128, 5 * FW], F32, tag="zf1")
            nc.gpsimd.memset(zf1[:], 0.0)
            s2 = state.tile([16, 64], MR, tag="s2")
            s_enc = state.tile([128, 2, 64], MR, tag="senc")
            GI = state.tile([128, 6, 64], F32, tag="gi")
            preds = state.tile([2, 4 * HOR], F32, tag="preds")

            # ---- state adapters: s1 = relu(a0 x); s2 = s1 + relu(ai s1) ----
            pa = psr.tile([16, 64], F32, tag="ps")
            mmr(pa[:], sv("a0t"), sv("xt"), start=True, stop=True)
            s1 = work.tile([16, 64], MR, tag="s1")
            nc.scalar.activation(s1[:], pa[:], AF.Relu, bias=svf("a0b"))
            pb = psr.tile([16, 64], F32, tag="ps")
            mmr(pb[:], sv("ait"), s1[:], start=True, stop=True)
            s1b = work.tile([16, 64], MR, tag="s1")
            nc.scalar.activation(s1b[:], pb[:], AF.Relu, bias=svf("aib"))
            nc.vector.tensor_add(s2[:], s1[:], s1b[:])

            # encoder hidden state
            h_cur = hpool.tile([128, 8], MR, tag="h")
            nc.vector.tensor_copy(h_cur[:], zf1[:, 0:8])

            def enc_step(t, h_prev):
                pg = psr.tile([128, 24], F32, tag="ps")
                for mc in range(6):
                    for kc in range(2):
                        mmr(pg[:, mc * 4:(mc + 1) * 4],
                            whh[:, kc * 768 + mc * 128:kc * 768 + (mc + 1) * 128],
                            h_prev[:, kc * 4:(kc + 1) * 4],
                            start=(kc == 0), stop=(kc == 1))
                gi_rz = GI[:, 0:4, t * 4:(t + 1) * 4]
                gi_n = GI[:, 4:6, t * 4:(t + 1) * 4]
                pre = work.tile([128, 16], F32, tag="g16")
                nc.vector.tensor_add(
                    pre[:].rearrange("p (c b) -> p c b", b=4), gi_rz,
                    pg[:, 0:16].rearrange("p (c b) -> p c b", b=4))
                rz = work.tile([128, 16], F32, tag="g16b")
                nc.scalar.activation(rz[:], pre[:], AF.Sigmoid)
                a1 = work.tile([128, 8], F32, tag="g8")
                nc.vector.tensor_add(a1[:], pg[:, 16:24], svf("bhhn"))
                a2 = work.tile([128, 8], F32, tag="g8b")
                nc.vector.tensor_mul(a2[:], rz[:, 0:8], a1[:])
                a3 = work.tile([128, 8], F32, tag="g8c")
                nc.vector.tensor_add(
                    a3[:].rearrange("p (c b) -> p c b", b=4),
                    a2[:].rearrange("p (c b) -> p c b", b=4), gi_n)
                nt = work.tile([128, 8], F32, tag="g8d")
                nc.scalar.activation(nt[:], a3[:], AF.Tanh)
                hmn = work.tile([128, 8], F32, tag="g8e")
                nc.vector.tensor_sub(hmn[:], h_prev[:], nt[:])
                zt = work.tile([128, 8], F32, tag="g8f")
                nc.vector.tensor_mul(zt[:], rz[:, 8:16], hmn[:])
                h_new = hpool.tile([128, 8], MR, tag="h")
                nc.vector.tensor_add(h_new[:], nt[:], zt[:])
                return h_new

            # ---- conv + features + GI, per group of 8 frames ----
            for g in range(NG):
                xin = xin_pool.tile([128, 6 * 392], MC, tag="xin")
                nc.sync.dma_start(xin[:], h_fr[g])

                # f1 in gutter-flat layout: data (f, ph, pw) at col
                # LEAD + f*64 + (ph+1)*8 + (pw+1); gutters stay zero.
                f1 = f1_pool.tile([128, 5, FW], mm_c2, tag="f1")
                nc.vector.tensor_copy(
                    f1[:].rearrange("p a b -> p (a b)"), zf1[:])

                for m in range(5):
                    msz = 128 if m < 4 else 64
                    p1 = ps1.tile([msz, 392], F32, tag="c1")
                    for k in range(6):
                        mmc(p1[:],
                            w1[:, k * 576 + m * 128:k * 576 + m * 128 + msz],
                            xin[:, k * 392:(k + 1) * 392],
                            start=(k == 0), stop=(k == 5))
                    dst = f1[0:msz, m, LEAD:LEAD + 8 * FB].rearrange(
                        "p (f a b) -> p f a b", a=8, b=8)[:, :, 1:8, 1:8]
                    nc.vector.tensor_copy(
                        dst, p1[:].rearrange("p (f a b) -> p f a b", a=7, b=7))

                # conv2: 9 taps x 5 ic-chunks, frame-half 0 on PE column half
                # 0 (out partitions 0:64) and frame-half 1 on column half 1
                # (out partitions 64:128) — the two run concurrently.
                p2 = ps2.tile([128, 256], F32, tag="c2", name=f"p2_{g}")
                for si, (dh, dw) in enumerate(SHIFTS):
                    s = dh * 3 + dw
                    delta = (dh - 1) * 8 + (dw - 1)
                    for k in range(5):
                        for h in (0, 1):
                            a = LEAD + h * 256 + delta
                            mmc2(p2[64 * h:64 * h + 64, :],
                                 w2[:, (s * 5 + k) * 64:(s * 5 + k + 1) * 64],
                                 f1[:, k, a:a + 256],
                                 start=(si == 0 and k == 0),
                                 stop=(si == len(SHIFTS) - 1 and k == 4),
                                 tile_position=(0, 64 * h)
                                 if C2_TILEPOS else None)

                # epilogue: relu(conv2 + B2) -> mean -> BN affine, per half
                # (half h lives on partitions 64h:64h+64 throughout)
                red = work.tile([128, 4], F32, tag="red")
                feats = work.tile([128, 4], MR, tag="feats")
                for h, b2n, psn, pshn in ((0, "b2t", "pscale", "pshift"),
                                          (1, "b2tB", "pscaleB", "pshiftB")):
                    r0 = 64 * h
                    pv = p2[r0:r0 + 64, :].rearrange(
                        "p (f a b) -> p f a b", a=8, b=8)[:, :, 1:8, 1:8]
                    b2v = svcf(b2n, h * 196, 196).rearrange(
                        "p (f a b) -> p f a b", a=7, b=7)
                    t0 = work.tile([128, 4, 7, 7], F32, tag="ep")
                    nc.vector.tensor_add(t0[r0:r0 + 64], pv, b2v)
                    t1 = work.tile([128, 4, 7, 7], F32, tag="ep2")
                    nc.vector.tensor_scalar_max(t1[r0:r0 + 64],
                                                t0[r0:r0 + 64], 0.0)
                    nc.vector.tensor_reduce(
                        red[r0:r0 + 64, :], t1[r0:r0 + 64],
                        axis=mybir.AxisListType.XY, op=mybir.AluOpType.add)
                    nc.scalar.activation(feats[r0:r0 + 64, :],
                                         red[r0:r0 + 64, :], AF.Identity,
                                         bias=svf(pshn), scale=svf(psn))

                # an: relu(an_w [s2; feats] + an_b); K split 64(feats)+16(s2),
                # N split per frame-half (feats halves live on different
                # partition ranges)
                for mc in range(2):
                    pan = psr.tile([128, FPG], F32, tag="ps")
                    for h, antn in ((0, "antf"), (1, "antfB")):
                        cs = slice(h * 4, h * 4 + 4)
                        gc = slice(g * FPG + h * 4, g * FPG + h * 4 + 4)
                        mmr(pan[:, cs], svc(antn, mc * 128, 128),
                            feats[64 * h:64 * h + 64, :],
                            start=True, stop=False)
                        mmr(pan[:, cs], svc("ants2", mc * 128, 128),
                            s2[:, gc], start=False, stop=True)
                    gcol = slice(g * FPG, (g + 1) * FPG)
                    nc.scalar.activation(s_enc[:, mc, gcol], pan[:], AF.Relu,
                                         bias=svcf("anb", mc, 1))

                # GI = w_ih @ s_enc + (b_ih + b_hh fold) for these 8 cols
                for mc in range(6):
                    pgi = psr.tile([128, FPG], F32, tag="ps")
                    for kc in range(2):
                        mmr(pgi[:],
                            wih[:, kc * 768 + mc * 128:kc * 768 + (mc + 1) * 128],
                            s_enc[:, kc, gcol],
                            start=(kc == 0), stop=(kc == 1))
                    nc.scalar.activation(GI[:, mc, gcol], pgi[:], AF.Identity,
                                         bias=svcf("bsgi", mc, 1))

                if debug and g == 0:
                    nc.sync.dma_start(h_df1[:], f1[:])

                # encoder steps that become ready after this group
                h_cur = enc_step(2 * g, h_cur)
                h_cur = enc_step(2 * g + 1, h_cur)

            if debug:
                nc.sync.dma_start(h_dsenc[:], s_enc[:])
                nc.sync.dma_start(h_dgi[:], GI[:])
                nc.sync.dma_start(h_dh[:], h_cur[:])

            # ---- decoder ----
            xi, hh = h_cur, h_cur
            for t in range(HOR):
                pg = psr.tile([128, 48], F32, tag="ps")
                # each PSUM accumulation group must complete before the next
                # one starts (interleaved groups break accumulation)
                for mc in range(6):
                    for kc in range(2):
                        mmr(pg[:, mc * 4:(mc + 1) * 4],
                            wih[:, kc * 768 + mc * 128:
                                kc * 768 + (mc + 1) * 128],
                            xi[:, kc * 4:(kc + 1) * 4],
                            start=(kc == 0), stop=(kc == 1))
                    for kc in range(2):
                        mmr(pg[:, 24 + mc * 4:24 + (mc + 1) * 4],
                            whh[:, kc * 768 + mc * 128:
                                kc * 768 + (mc + 1) * 128],
                            hh[:, kc * 4:(kc + 1) * 4],
                            start=(kc == 0), stop=(kc == 1))
                if debug and t == 0:
                    pgc = work.tile([128, 48], F32, tag="dbgpg")
                    nc.vector.tensor_copy(pgc[:], pg[:])
                    nc.sync.dma_start(h_dpg0[:], pgc[:])
                gisb = work.tile([128, 24], F32, tag="g24")
                nc.vector.tensor_add(gisb[:], pg[:, 0:24], svf("bsdec"))
                pre = work.tile([128, 16], F32, tag="g16")
                nc.vector.tensor_add(pre[:], gisb[:, 0:16], pg[:, 24:40])
                rz = work.tile([128, 16], F32, tag="g16b")
                nc.scalar.activation(rz[:], pre[:], AF.Sigmoid)
                a1 = work.tile([128, 8], F32, tag="g8")
                nc.vector.tensor_add(a1[:], pg[:, 40:48], svf("bhhn"))
                a2 = work.tile([128, 8], F32, tag="g8b")
                nc.vector.tensor_mul(a2[:], rz[:, 0:8], a1[:])
                a3 = work.tile([128, 8], F32, tag="g8c")
                nc.vector.tensor_add(a3[:], a2[:], gisb[:, 16:24])
                nt = work.tile([128, 8], F32, tag="g8d")
                nc.scalar.activation(nt[:], a3[:], AF.Tanh)
                hmn = work.tile([128, 8], F32, tag="g8e")
                nc.vector.tensor_sub(hmn[:], hh[:], nt[:])
                zt = work.tile([128, 8], F32, tag="g8f")
                nc.vector.tensor_mul(zt[:], rz[:, 8:16], hmn[:])
                hn = hpool.tile([128, 8], MR, tag="h")
                nc.vector.tensor_add(hn[:], nt[:], zt[:])

                pfi = psr.tile([128, 8], F32, tag="ps")
                for mc2 in range(2):
                    for kc2 in range(2):
                        mmr(pfi[:, mc2 * 4:(mc2 + 1) * 4],
                            fiw[:, kc2 * 256 + mc2 * 128:
                                kc2 * 256 + (mc2 + 1) * 128],
                            hn[:, kc2 * 4:(kc2 + 1) * 4],
                            start=(kc2 == 0), stop=(kc2 == 1))
                u1 = work.tile([128, 8], F32, tag="g8")
                nc.vector.tensor_add(u1[:], pfi[:], svf("fib"))
                u2 = work.tile([128, 8], F32, tag="g8b")
                nc.vector.tensor_scalar_max(u2[:], u1[:], 0.0)
                xr = hpool.tile([128, 8], MR, tag="xr")
                nc.vector.tensor_add(xr[:], hn[:], u2[:])

                pfn = psr.tile([2, 4], F32, tag="ps")
                for kc in range(2):
                    mmr(pfn[:], fnw[:, kc * 2:(kc + 1) * 2],
                        xr[:, kc * 4:(kc + 1) * 4],
                        start=(kc == 0), stop=(kc == 1))
                nc.scalar.activation(preds[:, t * 4:(t + 1) * 4], pfn[:],
                                     AF.Tanh, bias=svf("fnb"))
                if debug and t == 0:
                    nc.sync.dma_start(h_dhn0[:], hn[:])
                    nc.sync.dma_start(h_dxr0[:], xr[:])
                xi, hh = xr, hn

            nc.sync.dma_start(h_out[:], preds[:])

    nc.finalize()
    return nc


# ---------------- host-side data prep ----------------

def _prep_frames(frames):
    """frames (32,16,3,112,112) -> per-core [NG, 128, 2352] patch-T layout."""
    out = np.empty((NCORES, NG, 128, 6 * 392),
                   mybir.dt.np(_dt_of(MM_DT_CONV)))
    fr = np.ascontiguousarray(frames, np.float32)
    for c in range(NCORES):
        fb = fr[c * BPC:(c + 1) * BPC]  # (4, 16, 3, 112, 112)
        a = fb.reshape(BPC, L, 3, 7, 16, 7, 16)
        # -> [l, b, ch, kh, kw, ph, pw]
        a = a.transpose(1, 0, 2, 4, 6, 3, 5)
        a = a.reshape(L, BPC, 768, 49)
        a = a.reshape(NG, 2, BPC, 6, 128, 49)
        # -> [g, k, p, li, b, s]
        a = a.transpose(0, 3, 4, 1, 2, 5)
        a = a.reshape(NG, 6, 128, 392)
        a = a.transpose(0, 2, 1, 3)  # [g, p, k, 392]
        out[c] = a.reshape(NG, 128, 6 * 392)
    return out


def _prep_weights(iv):
    w = {}
    W1f = iv["cnn_w"].reshape(576, 768).astype(np.float32)
    w["w1"] = np.ascontiguousarray(
        W1f.T.reshape(6, 128, 576).transpose(1, 0, 2).reshape(128, 6 * 576))

    w2h = np.zeros((9, 5, 128, 64), np.float32)
    for dh in range(3):
        for dw in range(3):
            s = dh * 3 + dw
            T = iv["cnn1_w"][:, :, dh, dw].T.astype(np.float32)  # (576, 64)
            Tp = np.zeros((640, 64), np.float32)
            Tp[:576] = T
            w2h[s] = Tp.reshape(5, 128, 64)
    w["w2"] = np.ascontiguousarray(
        w2h.transpose(2, 0, 1, 3).reshape(128, 45 * 64)).astype(
            mybir.dt.np(_dt_of(MM_DT_C2)))

    rdt = mybir.dt.np(_dt_of(MM_DT_RNN))
    for name, key in (("wih", "w_ih"), ("whh", "w_hh")):
        T = iv[key].T.astype(np.float32)  # (256, 768)
        w[name] = np.ascontiguousarray(
            T.reshape(2, 128, 768).transpose(1, 0, 2).reshape(
                128, 1536)).astype(rdt)
    T = iv["fi_w"].T.astype(np.float32)  # (256, 256)
    w["fiw"] = np.ascontiguousarray(
        T.reshape(2, 128, 256).transpose(1, 0, 2).reshape(128, 512)).astype(rdt)
    T = iv["fn_w"].T.astype(np.float32)  # (256, 2)
    w["fnw"] = np.ascontiguousarray(
        T.reshape(2, 128, 2).transpose(1, 0, 2).reshape(128, 4)).astype(rdt)
    return w


def _prep_smalls(iv, x, core):
    smm = np.zeros((128, SMM_COLS), mybir.dt.np(_dt_of(MM_DT_RNN)))
    smb = np.zeros((128, SMB_COLS), np.float32)

    def put(name, arr):
        if name in SMM_LAYOUT:
            r0, rows, off, width = SMM_LAYOUT[name]
            dst = smm
        else:
            r0, rows, off, width = SMB_LAYOUT[name]
            dst = smb
        a = np.asarray(arr, np.float32).reshape(rows, width)
        dst[r0:r0 + rows, off:off + width] = a.astype(dst.dtype)

    # conv2 position-dependent bias fold (conv1 bias + cnn1_b)
    M = np.einsum("oiab,i->oab", iv["cnn1_w"], iv["cnn_b"]).astype(np.float32)
    B2 = np.zeros((64, 7, 7), np.float32)
    for ph in range(7):
        for pw in range(7):
            acc = iv["cnn1_b"].astype(np.float32).copy()
            for dh in range(3):
                for dw in range(3):
                    if 0 <= ph + dh - 1 <= 6 and 0 <= pw + dw - 1 <= 6:
                        acc = acc + M[:, dh, dw]
            B2[:, ph, pw] = acc
    put("b2t", np.tile(B2.reshape(64, 49), (1, FPG)))
    put("b2tB", np.tile(B2.reshape(64, 49), (1, FPG)))

    inv = iv["bn_g"] / np.sqrt(iv["bn_v"] + BN_EPS)
    put("pscale", (inv / 49.0)[:, None])
    put("pscaleB", (inv / 49.0)[:, None])
    put("pshift", (iv["bn_b"] - iv["bn_m"] * inv)[:, None])
    put("pshiftB", (iv["bn_b"] - iv["bn_m"] * inv)[:, None])

    xb = x[core * BPC:(core + 1) * BPC]  # (4, 16, 12)
    put("xt", xb.transpose(2, 1, 0).reshape(12, 64))

    put("a0t", iv["a0_w"].T)
    put("a0b", iv["a0_b"][:, None])
    put("ait", iv["ai_w"].T)
    put("aib", iv["ai_b"][:, None])
    put("ants2", iv["an_w"][:, 0:16].T)
    put("antf", iv["an_w"][:, 16:80].T)
    put("antfB", iv["an_w"][:, 16:80].T)
    put("anb", iv["an_b"].reshape(2, 128).T)

    bs = (iv["b_ih"] + iv["b_hh"]).astype(np.float32)
    bs[512:] = iv["b_ih"][512:]
    put("bsgi", bs.reshape(6, 128).T)
    put("bsdec", np.repeat(bs.reshape(6, 128).T, 4, axis=1))
    put("bhhn", np.repeat(iv["b_hh"][512:].reshape(2, 128).T, 4, axis=1))
    put("fib", np.repeat(iv["fi_b"].reshape(2, 128).T, 4, axis=1))
    put("fnb", iv["fn_b"][:, None])
    return smm, smb


def make_in_maps(inputs):
    iv = {k: np.asarray(v, np.float32) for k, v in inputs.items()}
    frames = iv["frames"]
    x = iv["x"]
    fr_all = _prep_frames(frames)
    w = _prep_weights(iv)
    in_maps = []
    for c in range(NCORES):
        smm, smb = _prep_smalls(iv, x, c)
        m = {"fr": np.ascontiguousarray(fr_all[c]), "smm": smm, "smb": smb}
        m.update(w)
        in_maps.append(m)
    return in_maps


_NC_CACHE = None


def get_nc():
    global _NC_CACHE
    if _NC_CACHE is None:
        _NC_CACHE = build_nc()
    return _NC_CACHE


def _install_ntff_hook():
    """The agent image's antenv lacks axon_hooks; synthesize it so
    run_bass_kernel_spmd(trace=True) can capture NTFF profiles."""
    try:
        from antenv.axon_hooks import get_axon_ntff_profile_hook  # noqa: F401
        return True
    except ImportError:
        pass
    try:
        import types
        import antenv
        if "/root/.axon_site" not in sys.path:
            sys.path.insert(0, "/root/.axon_site")
        from trn_agent_boot.trn_boot import _ntff_profile_via_ctypes
        hook = _ntff_profile_via_ctypes("/opt/axon/libaxon_pjrt.so")
        mod = types.ModuleType("antenv.axon_hooks")
        mod.get_axon_ntff_profile_hook = lambda: hook
        mod.set_axon_ntff_profile_hook = lambda h: None
        sys.modules["antenv.axon_hooks"] = mod
        antenv.axon_hooks = mod
        return hook is not None
    except Exception as e:  # pragma: no cover - profiling is best-effort
        print(f"ntff hook install failed: {e}")
        return False


def kernel(**inputs):
    global LAST_EXEC_NS, LAST_RESULTS
    nc = get_nc()
    in_maps = make_in_maps(inputs)
    trace = bool(int(os.environ.get("KERNEL_TRACE", "0")))
    if trace:
        trace = _install_ntff_hook()
    res = run_bass_kernel_spmd(nc, in_maps, core_ids=list(range(NCORES)),
                               trace=trace)
    LAST_RESULTS = res
    LAST_EXEC_NS = res.exec_time_ns
    outs = []
    for c in range(NCORES):
        o = res.results[c]["out"]  # (2, 40)
        outs.append(o.reshape(2, HOR, BPC).transpose(1, 2, 0)[:, :, None, :])
    return np.concatenate(outs, axis=1).astype(np.float32)


if __name__ == "__main__":
    nc = get_nc()
    print("built ok; instructions:",
          sum(len(bb.instructions) for bb in nc.main_func.blocks))
